# revision 15
# baseline (speedup 1.0000x reference)
"""Trainium2 Bass kernel for BatchTaskAlignedAssigner (YOLOv6 TAL assigner).

Full inputs (B=16) are sharded batch-wise across 8 NeuronCores (2 images each).
Per-core device algorithm (validated vs reference in numpy):
  anchor-major layout [128 partitions, free = 66 chunks x 64 gts] (8448 padded)
  - IoU / inside / align-metric elementwise chain on DVE (+ACT squares)
  - per-gt top-13 threshold via PE transpose to gt-major + DVE max8/match_replace
  - conflict resolution via per-anchor argmax code, gathers via masked sums
"""
import numpy as np

IMG = 2            # images per core
P = 8400           # anchors
CH = 66            # chunks of 128 anchors (padded to 8448)
CH2 = 33           # chunks per scores half
PM = 65 * 128      # 8320 anchors in full chunks
PT = P - PM        # 80 tail anchors in chunk 65
G = 64             # gts per image
C = 80             # classes
EPS = 1e-7
IOU_EPS = 1e-9
TINY = 1e-38

_CACHE = {}


def _build(taps=False):
    import concourse.bacc as bacc
    import concourse.tile as tile
    from concourse import mybir
    from concourse import bass_isa
    from contextlib import ExitStack

    dt = mybir.dt
    Alu = mybir.AluOpType
    Ax = mybir.AxisListType

    nc = bacc.Bacc("TRN2", target_bir_lowering=False, debug=False)

    # ---- DRAM I/O ----
    pb = nc.dram_tensor("pb", [IMG, P, 4], dt.float32, kind="ExternalInput").ap()
    ps = nc.dram_tensor("ps", [IMG, P, C], dt.float32, kind="ExternalInput").ap()
    pr = nc.dram_tensor("pr", [P, 4], dt.float32, kind="ExternalInput").ap()
    gl = nc.dram_tensor("gl", [IMG, G], dt.int32, kind="ExternalInput").ap()
    gb = nc.dram_tensor("gb", [IMG, G, 4], dt.float32, kind="ExternalInput").ap()
    fl = nc.dram_tensor("fl", [IMG, G], dt.float32, kind="ExternalInput").ap()
    olab = nc.dram_tensor("olab", [IMG, P], dt.int32, kind="ExternalOutput").ap()
    obb = nc.dram_tensor("obb", [IMG, P, 4], dt.float32, kind="ExternalOutput").ap()
    osc = nc.dram_tensor("osc", [IMG, P, C], dt.float32, kind="ExternalOutput").ap()
    ofg = nc.dram_tensor("ofg", [IMG, P], dt.uint8, kind="ExternalOutput").ap()
    if taps:
        dbs = nc.dram_tensor("dbs", [128, CH * G], dt.float32, kind="ExternalOutput").ap()
        diou = nc.dram_tensor("diou", [128, CH * G], dt.float32, kind="ExternalOutput").ap()
        dalg = nc.dram_tensor("dalg", [128, CH * G], dt.float32, kind="ExternalOutput").ap()
        dins = nc.dram_tensor("dins", [128, CH * G], dt.uint8, kind="ExternalOutput").ap()
        dmg = nc.dram_tensor("dmg", [128, CH * 128], dt.float32, kind="ExternalOutput").ap()
        dt13 = nc.dram_tensor("dt13", [128, 128], dt.float32, kind="ExternalOutput").ap()
        dpos = nc.dram_tensor("dpos", [128, CH * G], dt.float32, kind="ExternalOutput").ap()
        dmet = nc.dram_tensor("dmet", [128, CH * G], dt.float32, kind="ExternalOutput").ap()
        didx = nc.dram_tensor("didx", [128, 4 * CH2], dt.int16, kind="ExternalOutput").ap()
        dlw = nc.dram_tensor("dlw", [128, 4], dt.int32, kind="ExternalOutput").ap()

    F = CH * G  # 4224

    def bcC(t66):
        return t66[:, :, None].broadcast_to([128, CH, G])

    def bcG(t64):
        return t64[:, None, :].broadcast_to([128, CH, G])

    def load_anchor(tile_ap3, dram3):
        """dram3: [P, w] AP; tile_ap3: [128, CH, w] view. Pads chunk 65."""
        nc.sync.dma_start(
            tile_ap3[:, 0:65, :],
            dram3[0:PM].rearrange("(c p) x -> p c x", p=128, c=65))
        nc.vector.memset(tile_ap3[:, 65, :], 0.0)
        nc.sync.dma_start(tile_ap3[0:PT, 65, :], dram3[PM:P])

    def store_anchor(dram3, tile_ap3):
        nc.sync.dma_start(
            dram3[0:PM].rearrange("(c p) x -> p c x", p=128, c=65),
            tile_ap3[:, 0:65, :])
        nc.sync.dma_start(dram3[PM:P], tile_ap3[0:PT, 65, :])

    with tile.TileContext(nc) as tc, ExitStack() as ctx:
        cpool = ctx.enter_context(tc.tile_pool(name="const", bufs=1))
        big = ctx.enter_context(tc.tile_pool(name="big", bufs=1))
        small = ctx.enter_context(tc.tile_pool(name="small", bufs=1))
        psum = ctx.enter_context(tc.tile_pool(name="psum", bufs=2, space="PSUM"))
        psum1 = ctx.enter_context(tc.tile_pool(name="psum1", bufs=1, space="PSUM"))

        # ---- global constants ----
        ones = cpool.tile([128, 128], dt.float32)
        nc.gpsimd.memset(ones[:], 1.0)
        iden = cpool.tile([128, 128], dt.float32)
        nc.gpsimd.affine_select(iden[:], ones[:], pattern=[[1, 128]], base=0,
                                channel_multiplier=-1,
                                compare_op=Alu.is_equal, fill=0.0)
        clsio_i = cpool.tile([128, C], dt.int32)
        nc.gpsimd.iota(clsio_i[:], pattern=[[1, C]], base=0, channel_multiplier=0)
        clsio = cpool.tile([128, C], dt.float32)
        nc.vector.tensor_copy(clsio[:], clsio_i[:])
        gio_i = cpool.tile([128, G], dt.int32)
        nc.gpsimd.iota(gio_i[:], pattern=[[1, G]], base=0, channel_multiplier=0)
        giof = cpool.tile([128, G], dt.float32)
        nc.vector.tensor_copy(giof[:], gio_i[:])
        grev_i = cpool.tile([128, G], dt.int32)
        nc.gpsimd.iota(grev_i[:], pattern=[[-1, G]], base=G - 1, channel_multiplier=0)
        grev = cpool.tile([128, G], dt.float32)
        nc.vector.tensor_copy(grev[:], grev_i[:])
        infc = cpool.tile([128, 1], dt.float32)
        nc.gpsimd.memset(infc[:], 3.0e38)
        flcol = cpool.tile([128, 1], dt.float32)
        nc.sync.dma_start(flcol[:], fl.rearrange("i g -> (i g)")[:, None])
        flcol8 = cpool.tile([128, 1], dt.uint8)
        nc.vector.tensor_copy(flcol8[:], flcol[:])

        # priors (same for both images): anchor points
        ptst = cpool.tile([128, 2 * CH], dt.float32)
        load_anchor(ptst[:].rearrange("p (c x) -> p c x", x=2), pr[:, 0:2])
        pxv = ptst[:].rearrange("p (c x) -> p x c", x=2)[:, 0]     # [128, 66]
        pyv = ptst[:].rearrange("p (c x) -> p x c", x=2)[:, 1]

        mglob = big.tile([128, CH * 128], dt.float32, tag="mgsc", name="mglob")
        sA = cpool.tile([128, F], dt.float32)             # global scratch
        sB = cpool.tile([128, F], dt.float32)

        per = {}
        for i in range(IMG):
            per[i] = dict(
                iou=big.tile([128, F], dt.float32, tag=f"iou{i}", name=f"iou{i}"),
                alg=big.tile([128, F], dt.float32, tag=f"alg{i}", name=f"alg{i}"),
                ins=big.tile([128, F], dt.uint8, tag=f"ins{i}", name=f"ins{i}"),
            )

        gtt = {}

        # ================= phase A per image =================
        for i in range(IMG):
            pbt = small.tile([128, 4 * CH], dt.float32, tag="pbt")
            load_anchor(pbt[:].rearrange("p (c x) -> p c x", x=4), pb[i])
            pcv = pbt[:].rearrange("p (c x) -> p x c", x=4)
            px1, py1, px2, py2 = pcv[:, 0], pcv[:, 1], pcv[:, 2], pcv[:, 3]

            grow = small.tile([1, 4 * G], dt.float32, tag="grow")
            nc.sync.dma_start(grow[:], gb[i].rearrange("g x -> (g x)")[None, :])
            gbb = small.tile([128, 4 * G], dt.float32, tag=f"gbb{i}")
            nc.gpsimd.partition_broadcast(gbb[:], grow[:])
            gcv = gbb[:].rearrange("p (g x) -> p x g", x=4)
            gx1, gy1, gx2, gy2 = gcv[:, 0], gcv[:, 1], gcv[:, 2], gcv[:, 3]

            lrow = small.tile([1, G], dt.int32, tag="lrow")
            nc.sync.dma_start(lrow[:], gl[i][None, :])
            lbli = small.tile([128, G], dt.int32, tag="lbli")
            nc.gpsimd.partition_broadcast(lbli[:], lrow[:])
            lblf = small.tile([128, G], dt.float32, tag=f"lblf{i}")
            nc.vector.tensor_copy(lblf[:], lbli[:])

            # areas
            areag = small.tile([128, G], dt.float32, tag="areag")
            tw = small.tile([128, G], dt.float32, tag="tw64")
            nc.vector.tensor_tensor(tw[:], gx2, gx1, op=Alu.subtract)
            th = small.tile([128, G], dt.float32, tag="th64")
            nc.vector.tensor_tensor(th[:], gy2, gy1, op=Alu.subtract)
            nc.vector.tensor_tensor(areag[:], tw[:], th[:], op=Alu.mult)
            areap = small.tile([128, CH], dt.float32, tag="areap")
            tw2 = small.tile([128, CH], dt.float32, tag="tw66")
            nc.vector.tensor_tensor(tw2[:], px2, px1, op=Alu.subtract)
            th2 = small.tile([128, CH], dt.float32, tag="th66")
            nc.vector.tensor_tensor(th2[:], py2, py1, op=Alu.subtract)
            nc.vector.tensor_tensor(areap[:], tw2[:], th2[:], op=Alu.mult)

            gtt[i] = dict(gbb=gbb, lblf=lblf)

            # ---- label-gather indices (shared by both score halves) ----
            # idxs[p, k] = (k//4)*80 + lbl[16*(k%4) + p%16], int16, [128, 132]
            lw16 = small.tile([16, 4], dt.int32, tag="lw16")
            nc.sync.dma_start(lw16[:], gl[i].rearrange("(k q) -> q k", q=16))
            lw128 = small.tile([128, 4], dt.int32, tag="lw128")
            for r in range(8):
                nc.sync.dma_start(lw128[16 * r:16 * (r + 1), :], lw16[:])
            idx32 = small.tile([128, 4 * CH2], dt.int32, tag="idx32")
            nc.gpsimd.iota(idx32[:], pattern=[[C, CH2], [0, 4]], base=0,
                           channel_multiplier=0)
            nc.vector.tensor_tensor(
                idx32[:].rearrange("p (c k) -> p c k", k=4),
                idx32[:].rearrange("p (c k) -> p c k", k=4),
                lw128[:, None, :].broadcast_to([128, CH2, 4]),
                op=Alu.add)
            idx16 = small.tile([128, 4 * CH2], dt.int16, tag="idx16")
            nc.vector.tensor_copy(idx16[:], idx32[:])

            # ---- scores gather (two halves) ----
            bs = big.tile([128, F], dt.float32, tag="bufA", name=f"bs{i}")
            psre = ps[i][0:PM].rearrange("(c p) x -> p c x", p=128, c=65)
            for h in range(2):
                sco = big.tile([128, CH2 * C], dt.float32, tag="sc",
                               name=f"sco{i}{h}")
                sc3 = sco[:].rearrange("p (c x) -> p c x", x=C)
                if h == 0:
                    nc.sync.dma_start(sc3[:, :, :], psre[:, 0:33, :])
                else:
                    nc.sync.dma_start(sc3[:, 0:32, :], psre[:, 33:65, :])
                    nc.vector.memset(sc3[:, 32, :], 0.0)
                    nc.sync.dma_start(sc3[0:PT, 32, :], ps[i][PM:P])
                nc.gpsimd.ap_gather(bs[:, h * CH2 * G:(h + 1) * CH2 * G],
                                    sco[:], idx16[:], channels=128,
                                    num_elems=CH2 * C, d=1, num_idxs=CH2 * G)

            # ---- IoU / align / inside chain ----
            # temps: A=sA, B=sB, C=iou, D=alg, E=bufA(after bs dead)
            iou = per[i]["iou"]
            alg = per[i]["alg"]
            ins = per[i]["ins"]
            vA = sA[:].rearrange("p (c g) -> p c g", g=G)
            vB = sB[:].rearrange("p (c g) -> p c g", g=G)
            vC = iou[:].rearrange("p (c g) -> p c g", g=G)
            vD = alg[:].rearrange("p (c g) -> p c g", g=G)

            nc.vector.tensor_tensor(vC, bcC(px2), bcG(gx2), op=Alu.min)
            nc.vector.tensor_tensor(vD, bcC(px1), bcG(gx1), op=Alu.max)
            nc.vector.tensor_tensor(iou[:], iou[:], alg[:], op=Alu.subtract)  # ix
            nc.vector.tensor_tensor(vA, bcC(py2), bcG(gy2), op=Alu.min)
            nc.vector.tensor_tensor(vB, bcC(py1), bcG(gy1), op=Alu.max)
            nc.vector.tensor_tensor(sA[:], sA[:], sB[:], op=Alu.subtract)     # iy
            nc.vector.tensor_scalar(iou[:], iou[:], 0.0, None, op0=Alu.max)
            nc.vector.scalar_tensor_tensor(sB[:], sA[:], 0.0, iou[:],
                                           op0=Alu.max, op1=Alu.mult)   # inter
            nc.vector.tensor_tensor(vA, bcG(areag[:]), bcC(areap[:]), op=Alu.add)
            nc.vector.tensor_tensor(sA[:], sA[:], sB[:], op=Alu.subtract)  # union
            nc.vector.reciprocal_approx_fast(sA[:], sA[:])
            nc.vector.tensor_tensor(iou[:], sB[:], sA[:], op=Alu.mult)    # iou

            nc.scalar.square(sA[:], iou[:])            # o2 (ACT)
            nc.scalar.square(sB[:], sA[:])             # o4 (ACT)
            nc.vector.tensor_tensor(sA[:], sB[:], sA[:], op=Alu.mult)     # o6
            nc.vector.tensor_tensor(alg[:], sA[:], bs[:], op=Alu.mult)    # align

            etmp = big.tile([128, F], dt.float32, tag="bufA", name=f"etmp{i}")
            vE = etmp[:].rearrange("p (c g) -> p c g", g=G)
            nc.vector.tensor_tensor(vA, bcC(pxv), bcG(gx1), op=Alu.subtract)
            nc.vector.tensor_tensor(vB, bcG(gx2), bcC(pxv), op=Alu.subtract)
            nc.vector.tensor_tensor(sA[:], sA[:], sB[:], op=Alu.min)
            nc.vector.tensor_tensor(vB, bcC(pyv), bcG(gy1), op=Alu.subtract)
            nc.vector.tensor_tensor(vE, bcG(gy2), bcC(pyv), op=Alu.subtract)
            nc.vector.tensor_tensor(sB[:], sB[:], etmp[:], op=Alu.min)
            nc.vector.tensor_tensor(sA[:], sA[:], sB[:], op=Alu.min)      # mins
            nc.vector.tensor_scalar(sB[:], sA[:], IOU_EPS, None, op0=Alu.is_gt)
            nc.vector.tensor_copy(ins[:], sB[:])                           # u8
            nc.vector.tensor_tensor(sA[:], sB[:], alg[:], op=Alu.mult)    # metrics

            if taps and i == 0:
                nc.sync.dma_start(didx, idx16[:])
                nc.sync.dma_start(dlw, lw128[:])
                nc.sync.dma_start(dbs, bs[:])
                nc.sync.dma_start(diou, iou[:])
                nc.sync.dma_start(dalg, alg[:])
                nc.sync.dma_start(dins, ins[:])
                nc.sync.dma_start(dmet, sA[:])

            # ---- PE transpose metrics to gt-major ----
            groups = [(c0, min(8, CH - c0)) for c0 in range(0, CH, 8)]
            for (c0, n) in groups:
                pt = psum.tile([G, 8 * 128], dt.float32)
                for c in range(n):
                    nc.tensor.transpose(
                        pt[:, c * 128:(c + 1) * 128], vA[:, c0 + c, :], iden[:])
                nc.scalar.copy(mglob[i * G:(i + 1) * G, c0 * 128:(c0 + n) * 128],
                               pt[:, 0:n * 128])

        # ================= top-13 threshold =================
        m8a = small.tile([128, 8], dt.float32, tag="m8a")
        nc.vector.max(m8a[:], mglob[:])
        nc.vector.match_replace(mglob[:], m8a[:], mglob[:], imm_value=-1.0)
        m8b = small.tile([128, 8], dt.float32, tag="m8b")
        nc.vector.max(m8b[:], mglob[:])
        t13 = small.tile([128, 1], dt.float32, tag="t13")
        nc.vector.tensor_scalar(t13[:], m8b[:, 4:5], TINY, None, op0=Alu.max)
        t13s = small.tile([128, 1], dt.float32, tag="t13s")
        nc.vector.select(t13s[:], flcol8[:], t13[:], infc[:])
        ptr = psum1.tile([1, 128], dt.float32)
        nc.tensor.transpose(ptr[:], t13s[:], iden[:])
        t13r = small.tile([1, 128], dt.float32, tag="t13r")
        nc.scalar.copy(t13r[:], ptr[:])
        t13b = small.tile([128, 128], dt.float32, tag="t13b")
        nc.gpsimd.partition_broadcast(t13b[:], t13r[:])
        if taps:
            nc.sync.dma_start(dmg, mglob[:])
            nc.sync.dma_start(dt13, t13b[:])

        # ================= resolution + outputs per image =================
        for i in range(IMG):
            iou = per[i]["iou"]
            alg = per[i]["alg"]
            ins = per[i]["ins"]
            gbb = gtt[i]["gbb"]
            lblf = gtt[i]["lblf"]
            gcv = gbb[:].rearrange("p (g x) -> p x g", x=4)
            vA = sA[:].rearrange("p (c g) -> p c g", g=G)
            vB = sB[:].rearrange("p (c g) -> p c g", g=G)
            algv = alg[:].rearrange("p (c g) -> p c g", g=G)
            iouv = iou[:].rearrange("p (c g) -> p c g", g=G)

            ppre = big.tile([128, F], dt.float32, tag="bufA", name=f"ppre{i}")
            pv = ppre[:].rearrange("p (c g) -> p c g", g=G)
            tb = t13b[:, i * G:(i + 1) * G]
            nc.vector.tensor_tensor(pv, algv, bcG(tb), op=Alu.is_ge)
            nc.vector.tensor_tensor(ppre[:], ppre[:], ins[:], op=Alu.mult)

            fgpre = small.tile([128, CH], dt.float32, tag="fgpre")
            nc.vector.tensor_reduce(fgpre[:], pv, axis=Ax.X, op=Alu.add)
            colmax = small.tile([128, CH], dt.float32, tag="colmax")
            nc.vector.tensor_reduce(colmax[:], iouv, axis=Ax.X, op=Alu.max)
            nc.vector.tensor_tensor(vA, iouv, bcC(colmax[:]), op=Alu.is_equal)
            nc.vector.tensor_tensor(vA, vA, bcG(grev[:]), op=Alu.mult)
            argg = small.tile([128, CH], dt.float32, tag="argg")
            nc.vector.tensor_reduce(argg[:], vA, axis=Ax.X, op=Alu.max)
            nc.vector.tensor_scalar(argg[:], argg[:], -1.0, float(G - 1),
                                    op0=Alu.mult, op1=Alu.add)
            multi = small.tile([128, CH], dt.float32, tag="multi")
            nc.vector.tensor_scalar(multi[:], fgpre[:], 1.0, None, op0=Alu.is_gt)
            code = small.tile([128, CH], dt.float32, tag="code")
            nc.vector.scalar_tensor_tensor(code[:], argg[:], 1.0, multi[:],
                                           op0=Alu.add, op1=Alu.mult)
            nc.vector.tensor_scalar(code[:], code[:], -1.0, None, op0=Alu.add)
            nmask = small.tile([128, CH], dt.float32, tag="nmask")
            nc.vector.tensor_scalar(nmask[:], code[:], 0.0, None, op0=Alu.is_lt)
            nc.vector.tensor_tensor(vB, bcC(code[:]), bcG(giof[:]),
                                    op=Alu.is_equal)
            nc.vector.tensor_tensor(pv, pv, bcC(nmask[:]), op=Alu.mult)
            nc.vector.tensor_tensor(ppre[:], ppre[:], sB[:], op=Alu.add)  # pos
            pos = ppre
            pv = pos[:].rearrange("p (c g) -> p c g", g=G)

            if taps and i == 0:
                nc.sync.dma_start(dpos, pos[:])
            fg = small.tile([128, CH], dt.float32, tag=f"fg{i}")
            nc.vector.tensor_reduce(fg[:], pv, axis=Ax.X, op=Alu.add)

            nc.vector.tensor_tensor(sA[:], alg[:], pos[:], op=Alu.mult)   # am
            nc.vector.tensor_tensor(sB[:], iou[:], pos[:], op=Alu.mult)   # ovl*pos
            pac = small.tile([128, G], dt.float32, tag="pac")
            nc.vector.tensor_reduce(
                pac[:], sA[:].rearrange("p (c g) -> p g c", g=G), axis=Ax.X,
                op=Alu.max)
            paa = small.tile([128, G], dt.float32, tag="paa")
            nc.gpsimd.partition_all_reduce(paa[:], pac[:], channels=128,
                                           reduce_op=bass_isa.ReduceOp.max)
            poc = small.tile([128, G], dt.float32, tag="poc")
            nc.vector.tensor_reduce(
                poc[:], sB[:].rearrange("p (c g) -> p g c", g=G), axis=Ax.X,
                op=Alu.max)
            poa = small.tile([128, G], dt.float32, tag="poa")
            nc.gpsimd.partition_all_reduce(poa[:], poc[:], channels=128,
                                           reduce_op=bass_isa.ReduceOp.max)
            nc.vector.tensor_scalar(paa[:], paa[:], EPS, None, op0=Alu.add)
            cgr = small.tile([128, G], dt.float32, tag="cgr")
            nc.vector.reciprocal(cgr[:], paa[:])
            cg = small.tile([128, G], dt.float32, tag="cg")
            nc.vector.tensor_tensor(cg[:], poa[:], cgr[:], op=Alu.mult)

            # norm (am still in sA; in-place scale by cg then reduce)
            nc.vector.tensor_tensor(vA, vA, bcG(cg[:]), op=Alu.mult)
            normv = small.tile([128, CH], dt.float32, tag=f"normv{i}")
            nc.vector.tensor_reduce(normv[:], vA, axis=Ax.X, op=Alu.add)

            # labels
            fgf = small.tile([128, CH], dt.float32, tag="fgf")
            nc.vector.tensor_scalar(fgf[:], fg[:], 0.0, None, op0=Alu.is_gt)
            fgb8 = small.tile([128, CH], dt.uint8, tag="fgb8")
            nc.vector.tensor_copy(fgb8[:], fgf[:])
            nc.vector.tensor_tensor(vA, pv, bcG(lblf[:]), op=Alu.mult)
            lsum = small.tile([128, CH], dt.float32, tag="lsum")
            nc.vector.tensor_reduce(lsum[:], vA, axis=Ax.X, op=Alu.add)
            labf = small.tile([128, CH], dt.float32, tag=f"labf{i}")
            nc.vector.select(labf[:], fgb8[:], lsum[:],
                             lblf[:, 0:1].broadcast_to([128, CH]))

            # bboxes
            bbo = small.tile([128, 4 * CH], dt.float32, tag="bbo")
            bbv = bbo[:].rearrange("p (c x) -> p x c", x=4)
            for j in range(4):
                nc.vector.tensor_tensor(vA, pv, bcG(gcv[:, j]), op=Alu.mult)
                bsum = small.tile([128, CH], dt.float32, tag="bsum")
                nc.vector.tensor_reduce(bsum[:], vA, axis=Ax.X, op=Alu.add)
                nc.vector.select(bbv[:, j], fgb8[:], bsum[:],
                                 gbb[:, j:j + 1].broadcast_to([128, CH]))

            # labels cast
            labi = small.tile([128, CH], dt.int32, tag="labi")
            nc.vector.tensor_copy(labi[:], labf[:])

            # scores dense one-hot * norm
            sc_out = big.tile([128, CH * C], dt.float32, tag="mgsc",
                              name=f"sc_out{i}")
            scv = sc_out[:].rearrange("p (c x) -> p c x", x=C)
            nc.vector.tensor_tensor(
                scv,
                clsio[:, None, :].broadcast_to([128, CH, C]),
                labf[:, :, None].broadcast_to([128, CH, C]),
                op=Alu.is_equal)
            nc.vector.tensor_tensor(
                scv, scv,
                normv[:, :, None].broadcast_to([128, CH, C]),
                op=Alu.mult)

            # ---- outputs ----
            store_anchor(olab[i][:, None], labi[:, :, None])
            store_anchor(ofg[i][:, None], fgb8[:, :, None])
            store_anchor(obb[i], bbo[:].rearrange("p (c x) -> p c x", x=4))
            store_anchor(osc[i], scv)

    nc.compile()
    return nc


def _get_nc():
    if "nc" not in _CACHE:
        _CACHE["nc"] = _build()
    return _CACHE["nc"]


def kernel(pred_bboxes, pred_scores, priors, gt_labels, gt_bboxes, pad_bbox_flag):
    from concourse.bass_utils import run_bass_kernel_spmd

    pred_bboxes = np.ascontiguousarray(np.asarray(pred_bboxes, dtype=np.float32))
    pred_scores = np.ascontiguousarray(np.asarray(pred_scores, dtype=np.float32))
    priors = np.ascontiguousarray(np.asarray(priors, dtype=np.float32))
    lab_dt = np.asarray(gt_labels).dtype
    gli = np.ascontiguousarray(np.asarray(gt_labels).reshape(-1, G).astype(np.int32))
    gbf = np.ascontiguousarray(np.asarray(gt_bboxes, dtype=np.float32))
    flf = np.ascontiguousarray(
        np.asarray(pad_bbox_flag, dtype=np.float32).reshape(-1, G))

    B = pred_scores.shape[0]
    ncores = 8
    per = B // ncores
    assert per == IMG

    nc = _get_nc()
    in_maps = []
    for c in range(ncores):
        s = slice(c * per, (c + 1) * per)
        in_maps.append({
            "pb": pred_bboxes[s],
            "ps": pred_scores[s],
            "pr": priors,
            "gl": gli[s],
            "gb": gbf[s],
            "fl": flf[s],
        })
    res = run_bass_kernel_spmd(nc, in_maps, core_ids=list(range(ncores)))
    labs = np.concatenate([r["olab"] for r in res.results], axis=0)
    bbs = np.concatenate([r["obb"] for r in res.results], axis=0)
    scs = np.concatenate([r["osc"] for r in res.results], axis=0)
    fgs = np.concatenate([r["ofg"] for r in res.results], axis=0)
    return (labs.astype(lab_dt), bbs, scs, fgs.astype(bool))


# revision 18
# speedup vs baseline: 1.0587x; 1.0587x over previous
"""Trainium2 Bass kernel for BatchTaskAlignedAssigner (YOLOv6 TAL assigner).

Full inputs (B=16) are sharded batch-wise across 8 NeuronCores (2 images each).
Per-core device algorithm (validated vs reference in numpy):
  anchor-major layout [128 partitions, free = 66 chunks x 64 gts] (8448 padded)
  - IoU / inside / align-metric elementwise chain on DVE (+ACT squares)
  - per-gt top-13 threshold via PE transpose to gt-major + DVE max8/match_replace
  - conflict resolution via per-anchor argmax code, gathers via masked sums
"""
import numpy as np

IMG = 2            # images per core
P = 8400           # anchors
CH = 66            # chunks of 128 anchors (padded to 8448)
CH2 = 33           # chunks per scores half
PM = 65 * 128      # 8320 anchors in full chunks
PT = P - PM        # 80 tail anchors in chunk 65
G = 64             # gts per image
C = 80             # classes
EPS = 1e-7
IOU_EPS = 1e-9
TINY = 1e-38

_CACHE = {}


def _build(taps=False):
    import concourse.bacc as bacc
    import concourse.tile as tile
    from concourse import mybir
    from concourse import bass_isa
    from contextlib import ExitStack

    dt = mybir.dt
    Alu = mybir.AluOpType
    Ax = mybir.AxisListType

    nc = bacc.Bacc("TRN2", target_bir_lowering=False, debug=False)

    # ---- DRAM I/O ----
    pb = nc.dram_tensor("pb", [IMG, P, 4], dt.float32, kind="ExternalInput").ap()
    ps = nc.dram_tensor("ps", [IMG, P, C], dt.float32, kind="ExternalInput").ap()
    pr = nc.dram_tensor("pr", [P, 4], dt.float32, kind="ExternalInput").ap()
    gl = nc.dram_tensor("gl", [IMG, G], dt.int32, kind="ExternalInput").ap()
    gb = nc.dram_tensor("gb", [IMG, G, 4], dt.float32, kind="ExternalInput").ap()
    fl = nc.dram_tensor("fl", [IMG, G], dt.float32, kind="ExternalInput").ap()
    olab = nc.dram_tensor("olab", [IMG, P], dt.int32, kind="ExternalOutput").ap()
    obb = nc.dram_tensor("obb", [IMG, P, 4], dt.float32, kind="ExternalOutput").ap()
    osc = nc.dram_tensor("osc", [IMG, P, C], dt.float32, kind="ExternalOutput").ap()
    ofg = nc.dram_tensor("ofg", [IMG, P], dt.uint8, kind="ExternalOutput").ap()
    if taps:
        dbs = nc.dram_tensor("dbs", [128, CH * G], dt.float32, kind="ExternalOutput").ap()
        diou = nc.dram_tensor("diou", [128, CH * G], dt.float32, kind="ExternalOutput").ap()
        dalg = nc.dram_tensor("dalg", [128, CH * G], dt.float32, kind="ExternalOutput").ap()
        dins = nc.dram_tensor("dins", [128, CH * G], dt.uint8, kind="ExternalOutput").ap()
        dmg = nc.dram_tensor("dmg", [128, CH * 128], dt.float32, kind="ExternalOutput").ap()
        dt13 = nc.dram_tensor("dt13", [128, 128], dt.float32, kind="ExternalOutput").ap()
        dpos = nc.dram_tensor("dpos", [128, CH * G], dt.float32, kind="ExternalOutput").ap()
        dmet = nc.dram_tensor("dmet", [128, CH * G], dt.float32, kind="ExternalOutput").ap()
        didx = nc.dram_tensor("didx", [128, 4 * CH2], dt.int16, kind="ExternalOutput").ap()
        dlw = nc.dram_tensor("dlw", [128, 4], dt.int32, kind="ExternalOutput").ap()

    F = CH * G  # 4224

    def bcC(t66):
        return t66[:, :, None].broadcast_to([128, CH, G])

    def bcG(t64):
        return t64[:, None, :].broadcast_to([128, CH, G])

    def load_anchor(tile_ap3, dram3):
        """dram3: [P, w] AP; tile_ap3: [128, CH, w] view. Pads chunk 65."""
        nc.sync.dma_start(
            tile_ap3[:, 0:65, :],
            dram3[0:PM].rearrange("(c p) x -> p c x", p=128, c=65))
        nc.vector.memset(tile_ap3[:, 65, :], 0.0)
        nc.sync.dma_start(tile_ap3[0:PT, 65, :], dram3[PM:P])

    def store_anchor(dram3, tile_ap3):
        nc.sync.dma_start(
            dram3[0:PM].rearrange("(c p) x -> p c x", p=128, c=65),
            tile_ap3[:, 0:65, :])
        nc.sync.dma_start(dram3[PM:P], tile_ap3[0:PT, 65, :])

    with tile.TileContext(nc) as tc, ExitStack() as ctx:
        cpool = ctx.enter_context(tc.tile_pool(name="const", bufs=1))
        big = ctx.enter_context(tc.tile_pool(name="big", bufs=1))
        small = ctx.enter_context(tc.tile_pool(name="small", bufs=1))
        psum = ctx.enter_context(tc.tile_pool(name="psum", bufs=2, space="PSUM"))
        psum1 = ctx.enter_context(tc.tile_pool(name="psum1", bufs=1, space="PSUM"))

        # ---- global constants ----
        ones = cpool.tile([128, 128], dt.float32)
        nc.gpsimd.memset(ones[:], 1.0)
        iden = cpool.tile([128, 128], dt.float32)
        nc.gpsimd.affine_select(iden[:], ones[:], pattern=[[1, 128]], base=0,
                                channel_multiplier=-1,
                                compare_op=Alu.is_equal, fill=0.0)
        clsio_i = cpool.tile([128, C], dt.int32)
        nc.gpsimd.iota(clsio_i[:], pattern=[[1, C]], base=0, channel_multiplier=0)
        clsio = cpool.tile([128, C], dt.float32)
        nc.vector.tensor_copy(clsio[:], clsio_i[:])
        gio_i = cpool.tile([128, G], dt.int32)
        nc.gpsimd.iota(gio_i[:], pattern=[[1, G]], base=0, channel_multiplier=0)
        giof = cpool.tile([128, G], dt.float32)
        nc.vector.tensor_copy(giof[:], gio_i[:])
        grev_i = cpool.tile([128, G], dt.int32)
        nc.gpsimd.iota(grev_i[:], pattern=[[-1, G]], base=G - 1, channel_multiplier=0)
        grev = cpool.tile([128, G], dt.float32)
        nc.vector.tensor_copy(grev[:], grev_i[:])
        infc = cpool.tile([128, 1], dt.float32)
        nc.gpsimd.memset(infc[:], 3.0e38)
        flcol = cpool.tile([128, 1], dt.float32)
        nc.sync.dma_start(flcol[:], fl.rearrange("i g -> (i g)")[:, None])
        flcol8 = cpool.tile([128, 1], dt.uint8)
        nc.vector.tensor_copy(flcol8[:], flcol[:])

        # priors (same for both images): anchor points
        ptst = cpool.tile([128, 2 * CH], dt.float32)
        load_anchor(ptst[:].rearrange("p (c x) -> p c x", x=2), pr[:, 0:2])
        pxv = ptst[:].rearrange("p (c x) -> p x c", x=2)[:, 0]     # [128, 66]
        pyv = ptst[:].rearrange("p (c x) -> p x c", x=2)[:, 1]

        mglob = big.tile([128, CH * 128], dt.float32, tag="mgsc", name="mglob")
        sA = cpool.tile([128, F], dt.float32)             # global scratch
        sB = cpool.tile([128, F], dt.float32)

        per = {}
        for i in range(IMG):
            per[i] = dict(
                iou=big.tile([128, F], dt.float32, tag=f"iou{i}", name=f"iou{i}"),
                alg=big.tile([128, F], dt.float32, tag=f"alg{i}", name=f"alg{i}"),
                ins=big.tile([128, F], dt.uint8, tag=f"ins{i}", name=f"ins{i}"),
            )

        gtt = {}

        # ================= phase A per image =================
        for i in range(IMG):
            pbt = small.tile([128, 4 * CH], dt.float32, tag="pbt")
            load_anchor(pbt[:].rearrange("p (c x) -> p c x", x=4), pb[i])
            pcv = pbt[:].rearrange("p (c x) -> p x c", x=4)
            px1, py1, px2, py2 = pcv[:, 0], pcv[:, 1], pcv[:, 2], pcv[:, 3]

            grow = small.tile([1, 4 * G], dt.float32, tag="grow")
            nc.sync.dma_start(grow[:], gb[i].rearrange("g x -> (g x)")[None, :])
            gbb = small.tile([128, 4 * G], dt.float32, tag=f"gbb{i}")
            nc.gpsimd.partition_broadcast(gbb[:], grow[:])
            gcv = gbb[:].rearrange("p (g x) -> p x g", x=4)
            gx1, gy1, gx2, gy2 = gcv[:, 0], gcv[:, 1], gcv[:, 2], gcv[:, 3]

            lrow = small.tile([1, G], dt.int32, tag="lrow")
            nc.sync.dma_start(lrow[:], gl[i][None, :])
            lbli = small.tile([128, G], dt.int32, tag="lbli")
            nc.gpsimd.partition_broadcast(lbli[:], lrow[:])
            lblf = small.tile([128, G], dt.float32, tag=f"lblf{i}")
            nc.vector.tensor_copy(lblf[:], lbli[:])

            # areas
            areag = small.tile([128, G], dt.float32, tag="areag")
            tw = small.tile([128, G], dt.float32, tag="tw64")
            nc.vector.tensor_tensor(tw[:], gx2, gx1, op=Alu.subtract)
            th = small.tile([128, G], dt.float32, tag="th64")
            nc.vector.tensor_tensor(th[:], gy2, gy1, op=Alu.subtract)
            nc.vector.tensor_tensor(areag[:], tw[:], th[:], op=Alu.mult)
            areap = small.tile([128, CH], dt.float32, tag="areap")
            tw2 = small.tile([128, CH], dt.float32, tag="tw66")
            nc.vector.tensor_tensor(tw2[:], px2, px1, op=Alu.subtract)
            th2 = small.tile([128, CH], dt.float32, tag="th66")
            nc.vector.tensor_tensor(th2[:], py2, py1, op=Alu.subtract)
            nc.vector.tensor_tensor(areap[:], tw2[:], th2[:], op=Alu.mult)

            gtt[i] = dict(gbb=gbb, lblf=lblf)

            # ---- label-gather indices (shared by both score halves) ----
            # idxs[p, k] = (k//4)*80 + lbl[16*(k%4) + p%16], int16, [128, 132]
            lw16 = small.tile([16, 4], dt.int32, tag="lw16")
            nc.sync.dma_start(lw16[:], gl[i].rearrange("(k q) -> q k", q=16))
            lw128 = small.tile([128, 4], dt.int32, tag="lw128")
            for r in range(8):
                nc.sync.dma_start(lw128[16 * r:16 * (r + 1), :], lw16[:])
            idx32 = small.tile([128, 4 * CH2], dt.int32, tag="idx32")
            nc.gpsimd.iota(idx32[:], pattern=[[C, CH2], [0, 4]], base=0,
                           channel_multiplier=0)
            nc.vector.tensor_tensor(
                idx32[:].rearrange("p (c k) -> p c k", k=4),
                idx32[:].rearrange("p (c k) -> p c k", k=4),
                lw128[:, None, :].broadcast_to([128, CH2, 4]),
                op=Alu.add)
            idx16 = small.tile([128, 4 * CH2], dt.int16, tag="idx16")
            nc.vector.tensor_copy(idx16[:], idx32[:])

            # ---- scores gather (two halves) ----
            bs = big.tile([128, F], dt.float32, tag="bufA", name=f"bs{i}")
            psre = ps[i][0:PM].rearrange("(c p) x -> p c x", p=128, c=65)
            for h in range(2):
                sco = big.tile([128, CH2 * C], dt.float32, tag="sc",
                               name=f"sco{i}{h}")
                sc3 = sco[:].rearrange("p (c x) -> p c x", x=C)
                if h == 0:
                    nc.sync.dma_start(sc3[:, :, :], psre[:, 0:33, :])
                else:
                    nc.sync.dma_start(sc3[:, 0:32, :], psre[:, 33:65, :])
                    nc.vector.memset(sc3[:, 32, :], 0.0)
                    nc.sync.dma_start(sc3[0:PT, 32, :], ps[i][PM:P])
                nc.gpsimd.ap_gather(bs[:, h * CH2 * G:(h + 1) * CH2 * G],
                                    sco[:], idx16[:], channels=128,
                                    num_elems=CH2 * C, d=1, num_idxs=CH2 * G)

            # ---- IoU / align / inside chain ----
            # temps: A=sA, B=sB, C=iou, D=alg, E=bufA(after bs dead)
            iou = per[i]["iou"]
            alg = per[i]["alg"]
            ins = per[i]["ins"]
            vA = sA[:].rearrange("p (c g) -> p c g", g=G)
            vB = sB[:].rearrange("p (c g) -> p c g", g=G)
            vC = iou[:].rearrange("p (c g) -> p c g", g=G)
            vD = alg[:].rearrange("p (c g) -> p c g", g=G)

            nc.vector.tensor_tensor(vC, bcC(px2), bcG(gx2), op=Alu.min)
            nc.vector.tensor_tensor(vD, bcC(px1), bcG(gx1), op=Alu.max)
            nc.vector.tensor_tensor(iou[:], iou[:], alg[:], op=Alu.subtract)  # ix
            nc.vector.tensor_tensor(vA, bcC(py2), bcG(gy2), op=Alu.min)
            nc.vector.tensor_tensor(vB, bcC(py1), bcG(gy1), op=Alu.max)
            nc.vector.tensor_tensor(sA[:], sA[:], sB[:], op=Alu.subtract)     # iy
            nc.vector.tensor_scalar(iou[:], iou[:], 0.0, None, op0=Alu.max)
            nc.vector.scalar_tensor_tensor(sB[:], sA[:], 0.0, iou[:],
                                           op0=Alu.max, op1=Alu.mult)   # inter
            nc.vector.tensor_tensor(vA, bcG(areag[:]), bcC(areap[:]), op=Alu.add)
            nc.vector.tensor_tensor(sA[:], sA[:], sB[:], op=Alu.subtract)  # union
            nc.vector.reciprocal_approx_fast(sA[:], sA[:])
            nc.vector.tensor_tensor(iou[:], sB[:], sA[:], op=Alu.mult)    # iou

            nc.scalar.square(sA[:], iou[:])            # o2 (ACT)
            nc.scalar.square(sB[:], sA[:])             # o4 (ACT)
            nc.gpsimd.tensor_tensor(sA[:], sB[:], sA[:], op=Alu.mult)     # o6
            nc.vector.tensor_tensor(alg[:], sA[:], bs[:], op=Alu.mult)    # align

            etmp = big.tile([128, F], dt.float32, tag="bufA", name=f"etmp{i}")
            vE = etmp[:].rearrange("p (c g) -> p c g", g=G)
            nc.vector.tensor_tensor(vA, bcC(pxv), bcG(gx1), op=Alu.subtract)
            nc.gpsimd.tensor_tensor(vB, bcG(gx2), bcC(pxv), op=Alu.subtract)
            nc.vector.tensor_tensor(sA[:], sA[:], sB[:], op=Alu.min)
            nc.vector.tensor_tensor(vB, bcC(pyv), bcG(gy1), op=Alu.subtract)
            nc.gpsimd.tensor_tensor(vE, bcG(gy2), bcC(pyv), op=Alu.subtract)
            nc.vector.tensor_tensor(sB[:], sB[:], etmp[:], op=Alu.min)
            nc.vector.tensor_tensor(sA[:], sA[:], sB[:], op=Alu.min)      # mins
            nc.vector.tensor_scalar(sB[:], sA[:], IOU_EPS, None, op0=Alu.is_gt)
            nc.scalar.copy(ins[:], sB[:])                                  # u8
            nc.vector.tensor_tensor(sA[:], sB[:], alg[:], op=Alu.mult)    # metrics

            if taps and i == 0:
                nc.sync.dma_start(didx, idx16[:])
                nc.sync.dma_start(dlw, lw128[:])
                nc.sync.dma_start(dbs, bs[:])
                nc.sync.dma_start(diou, iou[:])
                nc.sync.dma_start(dalg, alg[:])
                nc.sync.dma_start(dins, ins[:])
                nc.sync.dma_start(dmet, sA[:])

            # ---- PE transpose metrics to gt-major ----
            groups = [(c0, min(8, CH - c0)) for c0 in range(0, CH, 8)]
            for (c0, n) in groups:
                pt = psum.tile([G, 8 * 128], dt.float32)
                for c in range(n):
                    nc.tensor.transpose(
                        pt[:, c * 128:(c + 1) * 128], vA[:, c0 + c, :], iden[:])
                nc.scalar.copy(mglob[i * G:(i + 1) * G, c0 * 128:(c0 + n) * 128],
                               pt[:, 0:n * 128])

        # ================= top-13 threshold =================
        m8a = small.tile([128, 8], dt.float32, tag="m8a")
        nc.vector.max(m8a[:], mglob[:])
        nc.vector.match_replace(mglob[:], m8a[:], mglob[:], imm_value=-1.0)
        m8b = small.tile([128, 8], dt.float32, tag="m8b")
        nc.vector.max(m8b[:], mglob[:])
        t13 = small.tile([128, 1], dt.float32, tag="t13")
        nc.vector.tensor_scalar(t13[:], m8b[:, 4:5], TINY, None, op0=Alu.max)
        t13s = small.tile([128, 1], dt.float32, tag="t13s")
        nc.vector.select(t13s[:], flcol8[:], t13[:], infc[:])
        ptr = psum1.tile([1, 128], dt.float32)
        nc.tensor.transpose(ptr[:], t13s[:], iden[:])
        t13r = small.tile([1, 128], dt.float32, tag="t13r")
        nc.scalar.copy(t13r[:], ptr[:])
        t13b = small.tile([128, 128], dt.float32, tag="t13b")
        nc.gpsimd.partition_broadcast(t13b[:], t13r[:])
        if taps:
            nc.sync.dma_start(dmg, mglob[:])
            nc.sync.dma_start(dt13, t13b[:])

        # ================= resolution + outputs per image =================
        for i in range(IMG):
            iou = per[i]["iou"]
            alg = per[i]["alg"]
            ins = per[i]["ins"]
            gbb = gtt[i]["gbb"]
            lblf = gtt[i]["lblf"]
            gcv = gbb[:].rearrange("p (g x) -> p x g", x=4)
            vA = sA[:].rearrange("p (c g) -> p c g", g=G)
            vB = sB[:].rearrange("p (c g) -> p c g", g=G)
            algv = alg[:].rearrange("p (c g) -> p c g", g=G)
            iouv = iou[:].rearrange("p (c g) -> p c g", g=G)

            ppre = big.tile([128, F], dt.float32, tag="bufA", name=f"ppre{i}")
            pv = ppre[:].rearrange("p (c g) -> p c g", g=G)
            tb = t13b[:, i * G:(i + 1) * G]
            nc.vector.tensor_tensor(pv, algv, bcG(tb), op=Alu.is_ge)
            nc.vector.tensor_tensor(ppre[:], ppre[:], ins[:], op=Alu.mult)

            fgpre = small.tile([128, CH], dt.float32, tag="fgpre")
            nc.vector.tensor_reduce(fgpre[:], pv, axis=Ax.X, op=Alu.add)
            colmax = small.tile([128, CH], dt.float32, tag="colmax")
            nc.vector.tensor_reduce(colmax[:], iouv, axis=Ax.X, op=Alu.max)
            nc.vector.tensor_tensor(vA, iouv, bcC(colmax[:]), op=Alu.is_equal)
            nc.gpsimd.tensor_tensor(vA, vA, bcG(grev[:]), op=Alu.mult)
            argg = small.tile([128, CH], dt.float32, tag="argg")
            nc.vector.tensor_reduce(argg[:], vA, axis=Ax.X, op=Alu.max)
            nc.vector.tensor_scalar(argg[:], argg[:], -1.0, float(G - 1),
                                    op0=Alu.mult, op1=Alu.add)
            multi = small.tile([128, CH], dt.float32, tag="multi")
            nc.vector.tensor_scalar(multi[:], fgpre[:], 1.0, None, op0=Alu.is_gt)
            code = small.tile([128, CH], dt.float32, tag="code")
            nc.vector.scalar_tensor_tensor(code[:], argg[:], 1.0, multi[:],
                                           op0=Alu.add, op1=Alu.mult)
            nc.vector.tensor_scalar(code[:], code[:], -1.0, None, op0=Alu.add)
            nmask = small.tile([128, CH], dt.float32, tag="nmask")
            nc.vector.tensor_scalar(nmask[:], code[:], 0.0, None, op0=Alu.is_lt)
            nc.vector.tensor_tensor(vB, bcC(code[:]), bcG(giof[:]),
                                    op=Alu.is_equal)
            nc.vector.tensor_tensor(pv, pv, bcC(nmask[:]), op=Alu.mult)
            nc.vector.tensor_tensor(ppre[:], ppre[:], sB[:], op=Alu.add)  # pos
            pos = ppre
            pv = pos[:].rearrange("p (c g) -> p c g", g=G)

            if taps and i == 0:
                nc.sync.dma_start(dpos, pos[:])
            fg = small.tile([128, CH], dt.float32, tag=f"fg{i}")
            nc.vector.tensor_reduce(fg[:], pv, axis=Ax.X, op=Alu.add)

            nc.vector.tensor_tensor(sA[:], alg[:], pos[:], op=Alu.mult)   # am
            nc.gpsimd.tensor_tensor(sB[:], iou[:], pos[:], op=Alu.mult)   # ovl*pos
            pac = small.tile([128, G], dt.float32, tag="pac")
            nc.vector.tensor_reduce(
                pac[:], sA[:].rearrange("p (c g) -> p g c", g=G), axis=Ax.X,
                op=Alu.max)
            paa = small.tile([128, G], dt.float32, tag="paa")
            nc.gpsimd.partition_all_reduce(paa[:], pac[:], channels=128,
                                           reduce_op=bass_isa.ReduceOp.max)
            poc = small.tile([128, G], dt.float32, tag="poc")
            nc.vector.tensor_reduce(
                poc[:], sB[:].rearrange("p (c g) -> p g c", g=G), axis=Ax.X,
                op=Alu.max)
            poa = small.tile([128, G], dt.float32, tag="poa")
            nc.gpsimd.partition_all_reduce(poa[:], poc[:], channels=128,
                                           reduce_op=bass_isa.ReduceOp.max)
            nc.vector.tensor_scalar(paa[:], paa[:], EPS, None, op0=Alu.add)
            cgr = small.tile([128, G], dt.float32, tag="cgr")
            nc.vector.reciprocal(cgr[:], paa[:])
            cg = small.tile([128, G], dt.float32, tag="cg")
            nc.vector.tensor_tensor(cg[:], poa[:], cgr[:], op=Alu.mult)

            # norm (am still in sA; in-place scale by cg then reduce)
            nc.vector.tensor_tensor(vA, vA, bcG(cg[:]), op=Alu.mult)
            normv = small.tile([128, CH], dt.float32, tag=f"normv{i}")
            nc.vector.tensor_reduce(normv[:], vA, axis=Ax.X, op=Alu.add)

            # labels
            fgf = small.tile([128, CH], dt.float32, tag="fgf")
            nc.vector.tensor_scalar(fgf[:], fg[:], 0.0, None, op0=Alu.is_gt)
            fgb8 = small.tile([128, CH], dt.uint8, tag="fgb8")
            nc.vector.tensor_copy(fgb8[:], fgf[:])
            nc.vector.tensor_tensor(vA, pv, bcG(lblf[:]), op=Alu.mult)
            lsum = small.tile([128, CH], dt.float32, tag="lsum")
            nc.vector.tensor_reduce(lsum[:], vA, axis=Ax.X, op=Alu.add)
            labf = small.tile([128, CH], dt.float32, tag=f"labf{i}")
            nc.vector.select(labf[:], fgb8[:], lsum[:],
                             lblf[:, 0:1].broadcast_to([128, CH]))

            # bboxes
            bbo = small.tile([128, 4 * CH], dt.float32, tag="bbo")
            bbv = bbo[:].rearrange("p (c x) -> p x c", x=4)
            for j in range(4):
                eng, tgt = (nc.vector, vA) if j % 2 == 0 else (nc.gpsimd, vB)
                eng.tensor_tensor(tgt, pv, bcG(gcv[:, j]), op=Alu.mult)
                bsum = small.tile([128, CH], dt.float32, tag="bsum")
                nc.vector.tensor_reduce(bsum[:], tgt, axis=Ax.X, op=Alu.add)
                nc.vector.select(bbv[:, j], fgb8[:], bsum[:],
                                 gbb[:, j:j + 1].broadcast_to([128, CH]))

            # labels cast
            labi = small.tile([128, CH], dt.int32, tag="labi")
            nc.vector.tensor_copy(labi[:], labf[:])

            # scores dense one-hot * norm
            sc_out = big.tile([128, CH * C], dt.float32, tag="mgsc",
                              name=f"sc_out{i}")
            scv = sc_out[:].rearrange("p (c x) -> p c x", x=C)
            nc.vector.tensor_tensor(
                scv,
                clsio[:, None, :].broadcast_to([128, CH, C]),
                labf[:, :, None].broadcast_to([128, CH, C]),
                op=Alu.is_equal)
            nc.vector.tensor_tensor(
                scv, scv,
                normv[:, :, None].broadcast_to([128, CH, C]),
                op=Alu.mult)

            # ---- outputs ----
            store_anchor(olab[i][:, None], labi[:, :, None])
            store_anchor(ofg[i][:, None], fgb8[:, :, None])
            store_anchor(obb[i], bbo[:].rearrange("p (c x) -> p c x", x=4))
            store_anchor(osc[i], scv)

    nc.compile()
    return nc


def _get_nc():
    if "nc" not in _CACHE:
        _CACHE["nc"] = _build()
    return _CACHE["nc"]


def kernel(pred_bboxes, pred_scores, priors, gt_labels, gt_bboxes, pad_bbox_flag):
    from concourse.bass_utils import run_bass_kernel_spmd

    pred_bboxes = np.ascontiguousarray(np.asarray(pred_bboxes, dtype=np.float32))
    pred_scores = np.ascontiguousarray(np.asarray(pred_scores, dtype=np.float32))
    priors = np.ascontiguousarray(np.asarray(priors, dtype=np.float32))
    lab_dt = np.asarray(gt_labels).dtype
    gli = np.ascontiguousarray(np.asarray(gt_labels).reshape(-1, G).astype(np.int32))
    gbf = np.ascontiguousarray(np.asarray(gt_bboxes, dtype=np.float32))
    flf = np.ascontiguousarray(
        np.asarray(pad_bbox_flag, dtype=np.float32).reshape(-1, G))

    B = pred_scores.shape[0]
    ncores = 8
    per = B // ncores
    assert per == IMG

    nc = _get_nc()
    in_maps = []
    for c in range(ncores):
        s = slice(c * per, (c + 1) * per)
        in_maps.append({
            "pb": pred_bboxes[s],
            "ps": pred_scores[s],
            "pr": priors,
            "gl": gli[s],
            "gb": gbf[s],
            "fl": flf[s],
        })
    res = run_bass_kernel_spmd(nc, in_maps, core_ids=list(range(ncores)))
    labs = np.concatenate([r["olab"] for r in res.results], axis=0)
    bbs = np.concatenate([r["obb"] for r in res.results], axis=0)
    scs = np.concatenate([r["osc"] for r in res.results], axis=0)
    fgs = np.concatenate([r["ofg"] for r in res.results], axis=0)
    return (labs.astype(lab_dt), bbs, scs, fgs.astype(bool))


# revision 22
# speedup vs baseline: 1.0818x; 1.0219x over previous
"""Trainium2 Bass kernel for BatchTaskAlignedAssigner (YOLOv6 TAL assigner).

Full inputs (B=16) are sharded batch-wise across 8 NeuronCores (2 images each).
Per-core device algorithm (validated vs reference in numpy):
  anchor-major layout [128 partitions, free = 66 chunks x 64 gts] (8448 padded)
  - IoU / inside / align-metric elementwise chain on DVE (+ACT squares)
  - per-gt top-13 threshold via PE transpose to gt-major + DVE max8/match_replace
  - conflict resolution via per-anchor argmax code, gathers via masked sums
"""
import numpy as np

IMG = 2            # images per core
P = 8400           # anchors
CH = 66            # chunks of 128 anchors (padded to 8448)
CH2 = 33           # chunks per scores half
PM = 65 * 128      # 8320 anchors in full chunks
PT = P - PM        # 80 tail anchors in chunk 65
G = 64             # gts per image
C = 80             # classes
EPS = 1e-7
IOU_EPS = 1e-9
TINY = 1e-38

_CACHE = {}


def _build(taps=False):
    import concourse.bacc as bacc
    import concourse.tile as tile
    from concourse import mybir
    from concourse import bass_isa
    from contextlib import ExitStack

    dt = mybir.dt
    Alu = mybir.AluOpType
    Ax = mybir.AxisListType

    nc = bacc.Bacc("TRN2", target_bir_lowering=False, debug=False)

    # ---- DRAM I/O ----
    pb = nc.dram_tensor("pb", [IMG, P, 4], dt.float32, kind="ExternalInput").ap()
    ps = nc.dram_tensor("ps", [IMG, P, C], dt.float32, kind="ExternalInput").ap()
    pr = nc.dram_tensor("pr", [P, 4], dt.float32, kind="ExternalInput").ap()
    gl = nc.dram_tensor("gl", [IMG, G], dt.int32, kind="ExternalInput").ap()
    gb = nc.dram_tensor("gb", [IMG, G, 4], dt.float32, kind="ExternalInput").ap()
    fl = nc.dram_tensor("fl", [IMG, G], dt.float32, kind="ExternalInput").ap()
    olab = nc.dram_tensor("olab", [IMG, P], dt.int32, kind="ExternalOutput").ap()
    obb = nc.dram_tensor("obb", [IMG, P, 4], dt.float32, kind="ExternalOutput").ap()
    osc = nc.dram_tensor("osc", [IMG, P, C], dt.float32, kind="ExternalOutput").ap()
    ofg = nc.dram_tensor("ofg", [IMG, P], dt.uint8, kind="ExternalOutput").ap()
    if taps:
        dbs = nc.dram_tensor("dbs", [128, CH * G], dt.float32, kind="ExternalOutput").ap()
        diou = nc.dram_tensor("diou", [128, CH * G], dt.float32, kind="ExternalOutput").ap()
        dalg = nc.dram_tensor("dalg", [128, CH * G], dt.float32, kind="ExternalOutput").ap()
        dins = nc.dram_tensor("dins", [128, CH * G], dt.uint8, kind="ExternalOutput").ap()
        dmg = nc.dram_tensor("dmg", [128, CH * 128], dt.float32, kind="ExternalOutput").ap()
        dt13 = nc.dram_tensor("dt13", [128, 128], dt.float32, kind="ExternalOutput").ap()
        dpos = nc.dram_tensor("dpos", [128, CH * G], dt.float32, kind="ExternalOutput").ap()
        dmet = nc.dram_tensor("dmet", [128, CH * G], dt.float32, kind="ExternalOutput").ap()
        didx = nc.dram_tensor("didx", [128, 4 * CH2], dt.int16, kind="ExternalOutput").ap()
        dlw = nc.dram_tensor("dlw", [128, 4], dt.int32, kind="ExternalOutput").ap()

    F = CH * G  # 4224

    def bcC(t66):
        return t66[:, :, None].broadcast_to([128, CH, G])

    def bcG(t64):
        return t64[:, None, :].broadcast_to([128, CH, G])

    def load_anchor(tile_ap3, dram3):
        """dram3: [P, w] AP; tile_ap3: [128, CH, w] view. Pads chunk 65."""
        nc.sync.dma_start(
            tile_ap3[:, 0:65, :],
            dram3[0:PM].rearrange("(c p) x -> p c x", p=128, c=65))
        nc.vector.memset(tile_ap3[:, 65, :], 0.0)
        nc.sync.dma_start(tile_ap3[0:PT, 65, :], dram3[PM:P])

    def store_anchor(dram3, tile_ap3):
        nc.sync.dma_start(
            dram3[0:PM].rearrange("(c p) x -> p c x", p=128, c=65),
            tile_ap3[:, 0:65, :])
        nc.sync.dma_start(dram3[PM:P], tile_ap3[0:PT, 65, :])

    with tile.TileContext(nc) as tc, ExitStack() as ctx:
        cpool = ctx.enter_context(tc.tile_pool(name="const", bufs=1))
        big = ctx.enter_context(tc.tile_pool(name="big", bufs=1))
        small = ctx.enter_context(tc.tile_pool(name="small", bufs=1))
        psum = ctx.enter_context(tc.tile_pool(name="psum", bufs=2, space="PSUM"))
        psum1 = ctx.enter_context(tc.tile_pool(name="psum1", bufs=1, space="PSUM"))

        # ---- global constants ----
        ones = cpool.tile([128, 128], dt.float32)
        nc.gpsimd.memset(ones[:], 1.0)
        iden = cpool.tile([128, 128], dt.float32)
        nc.gpsimd.affine_select(iden[:], ones[:], pattern=[[1, 128]], base=0,
                                channel_multiplier=-1,
                                compare_op=Alu.is_equal, fill=0.0)
        clsio_i = cpool.tile([128, C], dt.int32)
        nc.gpsimd.iota(clsio_i[:], pattern=[[1, C]], base=0, channel_multiplier=0)
        clsio = cpool.tile([128, C], dt.float32)
        nc.vector.tensor_copy(clsio[:], clsio_i[:])
        gio_i = cpool.tile([128, G], dt.int32)
        nc.gpsimd.iota(gio_i[:], pattern=[[1, G]], base=0, channel_multiplier=0)
        giof = cpool.tile([128, G], dt.float32)
        nc.vector.tensor_copy(giof[:], gio_i[:])
        grev_i = cpool.tile([128, G], dt.int32)
        nc.gpsimd.iota(grev_i[:], pattern=[[-1, G]], base=G - 1, channel_multiplier=0)
        grev = cpool.tile([128, G], dt.float32)
        nc.vector.tensor_copy(grev[:], grev_i[:])
        infc = cpool.tile([128, 1], dt.float32)
        nc.gpsimd.memset(infc[:], 3.0e38)
        flcol = cpool.tile([128, 1], dt.float32)
        nc.sync.dma_start(flcol[:], fl.rearrange("i g -> (i g)")[:, None])
        flcol8 = cpool.tile([128, 1], dt.uint8)
        nc.vector.tensor_copy(flcol8[:], flcol[:])

        # priors (same for both images): anchor points
        ptst = cpool.tile([128, 2 * CH], dt.float32)
        load_anchor(ptst[:].rearrange("p (c x) -> p c x", x=2), pr[:, 0:2])
        pxv = ptst[:].rearrange("p (c x) -> p x c", x=2)[:, 0]     # [128, 66]
        pyv = ptst[:].rearrange("p (c x) -> p x c", x=2)[:, 1]

        mglob = big.tile([128, CH * 128], dt.float32, tag="mgsc", name="mglob")
        sA = cpool.tile([128, F], dt.float32)             # global scratch
        sB = cpool.tile([128, F], dt.float32)

        per = {}
        for i in range(IMG):
            per[i] = dict(
                iou=big.tile([128, F], dt.float32, tag=f"iou{i}", name=f"iou{i}"),
                alg=big.tile([128, F], dt.float32, tag=f"alg{i}", name=f"alg{i}"),
                ins=big.tile([128, F], dt.uint8, tag=f"ins{i}", name=f"ins{i}"),
            )

        gtt = {}

        # ================= phase A per image =================
        for i in range(IMG):
            pbt = small.tile([128, 4 * CH], dt.float32, tag="pbt")
            load_anchor(pbt[:].rearrange("p (c x) -> p c x", x=4), pb[i])
            pcv = pbt[:].rearrange("p (c x) -> p x c", x=4)
            px1, py1, px2, py2 = pcv[:, 0], pcv[:, 1], pcv[:, 2], pcv[:, 3]

            grow = small.tile([1, 4 * G], dt.float32, tag="grow")
            nc.sync.dma_start(grow[:], gb[i].rearrange("g x -> (g x)")[None, :])
            gbb = small.tile([128, 4 * G], dt.float32, tag=f"gbb{i}")
            nc.gpsimd.partition_broadcast(gbb[:], grow[:])
            gcv = gbb[:].rearrange("p (g x) -> p x g", x=4)
            gx1, gy1, gx2, gy2 = gcv[:, 0], gcv[:, 1], gcv[:, 2], gcv[:, 3]

            lrow = small.tile([1, G], dt.int32, tag="lrow")
            nc.sync.dma_start(lrow[:], gl[i][None, :])
            lbli = small.tile([128, G], dt.int32, tag="lbli")
            nc.gpsimd.partition_broadcast(lbli[:], lrow[:])
            lblf = small.tile([128, G], dt.float32, tag=f"lblf{i}")
            nc.vector.tensor_copy(lblf[:], lbli[:])

            # areas
            areag = small.tile([128, G], dt.float32, tag="areag")
            tw = small.tile([128, G], dt.float32, tag="tw64")
            nc.vector.tensor_tensor(tw[:], gx2, gx1, op=Alu.subtract)
            th = small.tile([128, G], dt.float32, tag="th64")
            nc.vector.tensor_tensor(th[:], gy2, gy1, op=Alu.subtract)
            nc.vector.tensor_tensor(areag[:], tw[:], th[:], op=Alu.mult)
            areap = small.tile([128, CH], dt.float32, tag="areap")
            tw2 = small.tile([128, CH], dt.float32, tag="tw66")
            nc.vector.tensor_tensor(tw2[:], px2, px1, op=Alu.subtract)
            th2 = small.tile([128, CH], dt.float32, tag="th66")
            nc.vector.tensor_tensor(th2[:], py2, py1, op=Alu.subtract)
            nc.vector.tensor_tensor(areap[:], tw2[:], th2[:], op=Alu.mult)

            gtt[i] = dict(gbb=gbb, lblf=lblf)

            # ---- label-gather indices (shared by both score halves) ----
            # idxs[p, k] = (k//4)*80 + lbl[16*(k%4) + p%16], int16, [128, 132]
            lw16 = small.tile([16, 4], dt.int32, tag="lw16")
            nc.sync.dma_start(lw16[:], gl[i].rearrange("(k q) -> q k", q=16))
            lw128 = small.tile([128, 4], dt.int32, tag="lw128")
            for r in range(8):
                nc.sync.dma_start(lw128[16 * r:16 * (r + 1), :], lw16[:])
            idx32 = small.tile([128, 4 * CH2], dt.int32, tag="idx32")
            nc.gpsimd.iota(idx32[:], pattern=[[C, CH2], [0, 4]], base=0,
                           channel_multiplier=0)
            nc.vector.tensor_tensor(
                idx32[:].rearrange("p (c k) -> p c k", k=4),
                idx32[:].rearrange("p (c k) -> p c k", k=4),
                lw128[:, None, :].broadcast_to([128, CH2, 4]),
                op=Alu.add)
            idx16 = small.tile([128, 4 * CH2], dt.int16, tag="idx16")
            nc.vector.tensor_copy(idx16[:], idx32[:])

            # ---- scores gather (two halves) ----
            bs = big.tile([128, F], dt.float32, tag="bufA", name=f"bs{i}")
            psre = ps[i][0:PM].rearrange("(c p) x -> p c x", p=128, c=65)
            for h in range(2):
                sco = big.tile([128, CH2 * C], dt.float32, tag="sc",
                               name=f"sco{i}{h}")
                sc3 = sco[:].rearrange("p (c x) -> p c x", x=C)
                if h == 0:
                    nc.sync.dma_start(sc3[:, :, :], psre[:, 0:33, :])
                else:
                    nc.sync.dma_start(sc3[:, 0:32, :], psre[:, 33:65, :])
                    nc.vector.memset(sc3[:, 32, :], 0.0)
                    nc.sync.dma_start(sc3[0:PT, 32, :], ps[i][PM:P])
                nc.gpsimd.ap_gather(bs[:, h * CH2 * G:(h + 1) * CH2 * G],
                                    sco[:], idx16[:], channels=128,
                                    num_elems=CH2 * C, d=1, num_idxs=CH2 * G)

            # ---- IoU / align / inside chain ----
            # temps: A=sA, B=sB, C=iou, D=alg, E=bufA(after bs dead)
            iou = per[i]["iou"]
            alg = per[i]["alg"]
            ins = per[i]["ins"]
            vA = sA[:].rearrange("p (c g) -> p c g", g=G)
            vB = sB[:].rearrange("p (c g) -> p c g", g=G)
            vC = iou[:].rearrange("p (c g) -> p c g", g=G)
            vD = alg[:].rearrange("p (c g) -> p c g", g=G)

            nc.vector.tensor_tensor(vC, bcC(px2), bcG(gx2), op=Alu.min)
            nc.vector.tensor_tensor(vD, bcC(px1), bcG(gx1), op=Alu.max)
            nc.vector.tensor_tensor(iou[:], iou[:], alg[:], op=Alu.subtract)  # ix
            nc.vector.tensor_tensor(vA, bcC(py2), bcG(gy2), op=Alu.min)
            nc.vector.tensor_tensor(vB, bcC(py1), bcG(gy1), op=Alu.max)
            nc.vector.tensor_tensor(sA[:], sA[:], sB[:], op=Alu.subtract)     # iy
            nc.vector.tensor_scalar(iou[:], iou[:], 0.0, None, op0=Alu.max)
            nc.vector.scalar_tensor_tensor(sB[:], sA[:], 0.0, iou[:],
                                           op0=Alu.max, op1=Alu.mult)   # inter
            nc.vector.tensor_tensor(vA, bcG(areag[:]), bcC(areap[:]), op=Alu.add)
            nc.vector.tensor_tensor(sA[:], sA[:], sB[:], op=Alu.subtract)  # union
            nc.vector.reciprocal_approx_fast(sA[:], sA[:])
            nc.vector.tensor_tensor(iou[:], sB[:], sA[:], op=Alu.mult)    # iou

            nc.scalar.square(sA[:], iou[:])            # o2 (ACT)
            nc.scalar.square(sB[:], sA[:])             # o4 (ACT)
            nc.gpsimd.tensor_tensor(sA[:], sB[:], sA[:], op=Alu.mult)     # o6
            nc.vector.tensor_tensor(alg[:], sA[:], bs[:], op=Alu.mult)    # align

            etmp = big.tile([128, F], dt.float32, tag="bufA", name=f"etmp{i}")
            vE = etmp[:].rearrange("p (c g) -> p c g", g=G)
            nc.vector.tensor_tensor(vA, bcC(pxv), bcG(gx1), op=Alu.subtract)
            nc.gpsimd.tensor_tensor(vB, bcG(gx2), bcC(pxv), op=Alu.subtract)
            nc.vector.tensor_tensor(sA[:], sA[:], sB[:], op=Alu.min)
            nc.vector.tensor_tensor(vB, bcC(pyv), bcG(gy1), op=Alu.subtract)
            nc.gpsimd.tensor_tensor(vE, bcG(gy2), bcC(pyv), op=Alu.subtract)
            nc.vector.tensor_tensor(sB[:], sB[:], etmp[:], op=Alu.min)
            nc.vector.tensor_tensor(sA[:], sA[:], sB[:], op=Alu.min)      # mins
            nc.vector.tensor_scalar(sB[:], sA[:], IOU_EPS, None, op0=Alu.is_gt)
            nc.scalar.copy(ins[:], sB[:])                                  # u8
            nc.vector.tensor_tensor(sA[:], sB[:], alg[:], op=Alu.mult)    # metrics

            if taps and i == 0:
                nc.sync.dma_start(didx, idx16[:])
                nc.sync.dma_start(dlw, lw128[:])
                nc.sync.dma_start(dbs, bs[:])
                nc.sync.dma_start(diou, iou[:])
                nc.sync.dma_start(dalg, alg[:])
                nc.sync.dma_start(dins, ins[:])
                nc.sync.dma_start(dmet, sA[:])

            # ---- PE transpose metrics to gt-major ----
            groups = [(c0, min(8, CH - c0)) for c0 in range(0, CH, 8)]
            for (c0, n) in groups:
                pt = psum.tile([G, 8 * 128], dt.float32)
                for c in range(n):
                    nc.tensor.transpose(
                        pt[:, c * 128:(c + 1) * 128], vA[:, c0 + c, :], iden[:])
                nc.scalar.copy(mglob[i * G:(i + 1) * G, c0 * 128:(c0 + n) * 128],
                               pt[:, 0:n * 128])

        # ================= top-13 threshold =================
        m8a = small.tile([128, 8], dt.float32, tag="m8a")
        nc.vector.max(m8a[:], mglob[:])
        nc.vector.match_replace(mglob[:], m8a[:], mglob[:], imm_value=-1.0)
        m8b = small.tile([128, 8], dt.float32, tag="m8b")
        nc.vector.max(m8b[:], mglob[:])
        t13 = small.tile([128, 1], dt.float32, tag="t13")
        nc.vector.tensor_scalar(t13[:], m8b[:, 4:5], TINY, None, op0=Alu.max)
        t13s = small.tile([128, 1], dt.float32, tag="t13s")
        nc.vector.select(t13s[:], flcol8[:], t13[:], infc[:])
        ptr = psum1.tile([1, 128], dt.float32)
        nc.tensor.transpose(ptr[:], t13s[:], iden[:])
        t13r = small.tile([1, 128], dt.float32, tag="t13r")
        nc.scalar.copy(t13r[:], ptr[:])
        t13b = small.tile([128, 128], dt.float32, tag="t13b")
        nc.gpsimd.partition_broadcast(t13b[:], t13r[:])
        if taps:
            nc.sync.dma_start(dmg, mglob[:])
            nc.sync.dma_start(dt13, t13b[:])

        # ================= resolution + outputs per image =================
        for i in range(IMG):
            iou = per[i]["iou"]
            alg = per[i]["alg"]
            ins = per[i]["ins"]
            gbb = gtt[i]["gbb"]
            lblf = gtt[i]["lblf"]
            gcv = gbb[:].rearrange("p (g x) -> p x g", x=4)
            vA = sA[:].rearrange("p (c g) -> p c g", g=G)
            vB = sB[:].rearrange("p (c g) -> p c g", g=G)
            algv = alg[:].rearrange("p (c g) -> p c g", g=G)
            iouv = iou[:].rearrange("p (c g) -> p c g", g=G)

            ppre = big.tile([128, F], dt.float32, tag="bufA", name=f"ppre{i}")
            pv = ppre[:].rearrange("p (c g) -> p c g", g=G)
            tb = t13b[:, i * G:(i + 1) * G]
            nc.vector.tensor_tensor(pv, algv, bcG(tb), op=Alu.is_ge)
            nc.vector.tensor_tensor(ppre[:], ppre[:], ins[:], op=Alu.mult)

            fgpre = small.tile([128, CH], dt.float32, tag="fgpre")
            nc.vector.tensor_reduce(fgpre[:], pv, axis=Ax.X, op=Alu.add)
            colmax = small.tile([128, CH], dt.float32, tag="colmax")
            nc.vector.tensor_reduce(colmax[:], iouv, axis=Ax.X, op=Alu.max)
            nc.vector.tensor_tensor(vA, iouv, bcC(colmax[:]), op=Alu.is_equal)
            nc.gpsimd.tensor_tensor(vA, vA, bcG(grev[:]), op=Alu.mult)
            argg = small.tile([128, CH], dt.float32, tag="argg")
            nc.vector.tensor_reduce(argg[:], vA, axis=Ax.X, op=Alu.max)
            nc.vector.tensor_scalar(argg[:], argg[:], -1.0, float(G - 1),
                                    op0=Alu.mult, op1=Alu.add)
            multi = small.tile([128, CH], dt.float32, tag="multi")
            nc.vector.tensor_scalar(multi[:], fgpre[:], 1.0, None, op0=Alu.is_gt)
            code = small.tile([128, CH], dt.float32, tag="code")
            nc.vector.scalar_tensor_tensor(code[:], argg[:], 1.0, multi[:],
                                           op0=Alu.add, op1=Alu.mult)
            nc.vector.tensor_scalar(code[:], code[:], -1.0, None, op0=Alu.add)
            nmask = small.tile([128, CH], dt.float32, tag="nmask")
            nc.vector.tensor_scalar(nmask[:], code[:], 0.0, None, op0=Alu.is_lt)
            nc.vector.tensor_tensor(vB, bcC(code[:]), bcG(giof[:]),
                                    op=Alu.is_equal)
            nc.vector.tensor_tensor(pv, pv, bcC(nmask[:]), op=Alu.mult)
            nc.vector.tensor_tensor(ppre[:], ppre[:], sB[:], op=Alu.add)  # pos
            pos = ppre
            pv = pos[:].rearrange("p (c g) -> p c g", g=G)

            if taps and i == 0:
                nc.sync.dma_start(dpos, pos[:])
            fg = small.tile([128, CH], dt.float32, tag=f"fg{i}")
            nc.vector.tensor_reduce(fg[:], pv, axis=Ax.X, op=Alu.add)

            nc.vector.tensor_tensor(sA[:], alg[:], pos[:], op=Alu.mult)   # am
            nc.gpsimd.tensor_tensor(sB[:], iou[:], pos[:], op=Alu.mult)   # ovl*pos
            pac = small.tile([128, G], dt.float32, tag="pac")
            nc.vector.tensor_reduce(
                pac[:], sA[:].rearrange("p (c g) -> p g c", g=G), axis=Ax.X,
                op=Alu.max)
            paa = small.tile([128, G], dt.float32, tag="paa")
            nc.gpsimd.partition_all_reduce(paa[:], pac[:], channels=128,
                                           reduce_op=bass_isa.ReduceOp.max)
            poc = small.tile([128, G], dt.float32, tag="poc")
            nc.vector.tensor_reduce(
                poc[:], sB[:].rearrange("p (c g) -> p g c", g=G), axis=Ax.X,
                op=Alu.max)
            poa = small.tile([128, G], dt.float32, tag="poa")
            nc.gpsimd.partition_all_reduce(poa[:], poc[:], channels=128,
                                           reduce_op=bass_isa.ReduceOp.max)
            nc.vector.tensor_scalar(paa[:], paa[:], EPS, None, op0=Alu.add)
            cgr = small.tile([128, G], dt.float32, tag="cgr")
            nc.vector.reciprocal(cgr[:], paa[:])
            cg = small.tile([128, G], dt.float32, tag="cg")
            nc.vector.tensor_tensor(cg[:], poa[:], cgr[:], op=Alu.mult)

            # norm (am still in sA; in-place scale by cg then reduce)
            nc.vector.tensor_tensor(vA, vA, bcG(cg[:]), op=Alu.mult)
            normv = small.tile([128, CH], dt.float32, tag=f"normv{i}")
            nc.vector.tensor_reduce(normv[:], vA, axis=Ax.X, op=Alu.add)

            # labels
            fgf = small.tile([128, CH], dt.float32, tag="fgf")
            nc.vector.tensor_scalar(fgf[:], fg[:], 0.0, None, op0=Alu.is_gt)
            fgb8 = small.tile([128, CH], dt.uint8, tag="fgb8")
            nc.vector.tensor_copy(fgb8[:], fgf[:])
            nc.vector.tensor_tensor(vA, pv, bcG(lblf[:]), op=Alu.mult)
            lsum = small.tile([128, CH], dt.float32, tag="lsum")
            nc.vector.tensor_reduce(lsum[:], vA, axis=Ax.X, op=Alu.add)
            labf = small.tile([128, CH], dt.float32, tag=f"labf{i}")
            nc.vector.select(labf[:], fgb8[:], lsum[:],
                             lblf[:, 0:1].broadcast_to([128, CH]))

            # bboxes
            bbo = small.tile([128, 4 * CH], dt.float32, tag="bbo")
            bbv = bbo[:].rearrange("p (c x) -> p x c", x=4)
            for j in range(4):
                eng, tgt = (nc.vector, vA) if j % 2 == 0 else (nc.gpsimd, vB)
                eng.tensor_tensor(tgt, pv, bcG(gcv[:, j]), op=Alu.mult)
                bsum = small.tile([128, CH], dt.float32, tag="bsum")
                nc.vector.tensor_reduce(bsum[:], tgt, axis=Ax.X, op=Alu.add)
                nc.vector.select(bbv[:, j], fgb8[:], bsum[:],
                                 gbb[:, j:j + 1].broadcast_to([128, CH]))

            # labels cast
            labi = small.tile([128, CH], dt.int32, tag="labi")
            nc.vector.tensor_copy(labi[:], labf[:])

            # scores dense one-hot * norm
            sc_out = big.tile([128, CH * C], dt.float32, tag="mgsc",
                              name=f"sc_out{i}")
            scv = sc_out[:].rearrange("p (c x) -> p c x", x=C)
            nc.vector.tensor_tensor(
                scv,
                clsio[:, None, :].broadcast_to([128, CH, C]),
                labf[:, :, None].broadcast_to([128, CH, C]),
                op=Alu.is_equal)
            nc.gpsimd.tensor_tensor(
                scv, scv,
                normv[:, :, None].broadcast_to([128, CH, C]),
                op=Alu.mult)

            # ---- outputs ----
            store_anchor(olab[i][:, None], labi[:, :, None])
            store_anchor(ofg[i][:, None], fgb8[:, :, None])
            store_anchor(obb[i], bbo[:].rearrange("p (c x) -> p c x", x=4))
            store_anchor(osc[i], scv)

    nc.compile()
    return nc


def _get_nc():
    if "nc" not in _CACHE:
        _CACHE["nc"] = _build()
    return _CACHE["nc"]


def kernel(pred_bboxes, pred_scores, priors, gt_labels, gt_bboxes, pad_bbox_flag):
    from concourse.bass_utils import run_bass_kernel_spmd

    pred_bboxes = np.ascontiguousarray(np.asarray(pred_bboxes, dtype=np.float32))
    pred_scores = np.ascontiguousarray(np.asarray(pred_scores, dtype=np.float32))
    priors = np.ascontiguousarray(np.asarray(priors, dtype=np.float32))
    lab_dt = np.asarray(gt_labels).dtype
    gli = np.ascontiguousarray(np.asarray(gt_labels).reshape(-1, G).astype(np.int32))
    gbf = np.ascontiguousarray(np.asarray(gt_bboxes, dtype=np.float32))
    flf = np.ascontiguousarray(
        np.asarray(pad_bbox_flag, dtype=np.float32).reshape(-1, G))

    B = pred_scores.shape[0]
    ncores = 8
    per = B // ncores
    assert per == IMG

    nc = _get_nc()
    in_maps = []
    for c in range(ncores):
        s = slice(c * per, (c + 1) * per)
        in_maps.append({
            "pb": pred_bboxes[s],
            "ps": pred_scores[s],
            "pr": priors,
            "gl": gli[s],
            "gb": gbf[s],
            "fl": flf[s],
        })
    res = run_bass_kernel_spmd(nc, in_maps, core_ids=list(range(ncores)))
    labs = np.concatenate([r["olab"] for r in res.results], axis=0)
    bbs = np.concatenate([r["obb"] for r in res.results], axis=0)
    scs = np.concatenate([r["osc"] for r in res.results], axis=0)
    fgs = np.concatenate([r["ofg"] for r in res.results], axis=0)
    return (labs.astype(lab_dt), bbs, scs, fgs.astype(bool))


# revision 23
# speedup vs baseline: 1.0885x; 1.0061x over previous
"""Trainium2 Bass kernel for BatchTaskAlignedAssigner (YOLOv6 TAL assigner).

Full inputs (B=16) are sharded batch-wise across 8 NeuronCores (2 images each).
Per-core device algorithm (validated vs reference in numpy):
  anchor-major layout [128 partitions, free = 66 chunks x 64 gts] (8448 padded)
  - IoU / inside / align-metric elementwise chain on DVE (+ACT squares)
  - per-gt top-13 threshold via PE transpose to gt-major + DVE max8/match_replace
  - conflict resolution via per-anchor argmax code, gathers via masked sums
"""
import numpy as np

IMG = 2            # images per core
P = 8400           # anchors
CH = 66            # chunks of 128 anchors (padded to 8448)
CH2 = 33           # chunks per scores half
PM = 65 * 128      # 8320 anchors in full chunks
PT = P - PM        # 80 tail anchors in chunk 65
G = 64             # gts per image
C = 80             # classes
EPS = 1e-7
IOU_EPS = 1e-9
TINY = 1e-38

_CACHE = {}


def _build(taps=False):
    import concourse.bacc as bacc
    import concourse.tile as tile
    from concourse import mybir
    from concourse import bass_isa
    from contextlib import ExitStack

    dt = mybir.dt
    Alu = mybir.AluOpType
    Ax = mybir.AxisListType

    nc = bacc.Bacc("TRN2", target_bir_lowering=False, debug=False)

    # ---- DRAM I/O ----
    pb = nc.dram_tensor("pb", [IMG, P, 4], dt.float32, kind="ExternalInput").ap()
    ps = nc.dram_tensor("ps", [IMG, P, C], dt.float32, kind="ExternalInput").ap()
    pr = nc.dram_tensor("pr", [P, 4], dt.float32, kind="ExternalInput").ap()
    gl = nc.dram_tensor("gl", [IMG, G], dt.int32, kind="ExternalInput").ap()
    gb = nc.dram_tensor("gb", [IMG, G, 4], dt.float32, kind="ExternalInput").ap()
    fl = nc.dram_tensor("fl", [IMG, G], dt.float32, kind="ExternalInput").ap()
    olab = nc.dram_tensor("olab", [IMG, P], dt.int32, kind="ExternalOutput").ap()
    obb = nc.dram_tensor("obb", [IMG, P, 4], dt.float32, kind="ExternalOutput").ap()
    osc = nc.dram_tensor("osc", [IMG, P, C], dt.float32, kind="ExternalOutput").ap()
    ofg = nc.dram_tensor("ofg", [IMG, P], dt.uint8, kind="ExternalOutput").ap()
    if taps:
        dbs = nc.dram_tensor("dbs", [128, CH * G], dt.float32, kind="ExternalOutput").ap()
        diou = nc.dram_tensor("diou", [128, CH * G], dt.float32, kind="ExternalOutput").ap()
        dalg = nc.dram_tensor("dalg", [128, CH * G], dt.float32, kind="ExternalOutput").ap()
        dins = nc.dram_tensor("dins", [128, CH * G], dt.uint8, kind="ExternalOutput").ap()
        dmg = nc.dram_tensor("dmg", [128, CH * 128], dt.float32, kind="ExternalOutput").ap()
        dt13 = nc.dram_tensor("dt13", [128, 128], dt.float32, kind="ExternalOutput").ap()
        dpos = nc.dram_tensor("dpos", [128, CH * G], dt.float32, kind="ExternalOutput").ap()
        dmet = nc.dram_tensor("dmet", [128, CH * G], dt.float32, kind="ExternalOutput").ap()
        didx = nc.dram_tensor("didx", [128, 4 * CH2], dt.int16, kind="ExternalOutput").ap()
        dlw = nc.dram_tensor("dlw", [128, 4], dt.int32, kind="ExternalOutput").ap()

    F = CH * G  # 4224

    def bcC(t66):
        return t66[:, :, None].broadcast_to([128, CH, G])

    def bcG(t64):
        return t64[:, None, :].broadcast_to([128, CH, G])

    def load_anchor(tile_ap3, dram3):
        """dram3: [P, w] AP; tile_ap3: [128, CH, w] view. Pads chunk 65."""
        nc.sync.dma_start(
            tile_ap3[:, 0:65, :],
            dram3[0:PM].rearrange("(c p) x -> p c x", p=128, c=65))
        nc.vector.memset(tile_ap3[:, 65, :], 0.0)
        nc.sync.dma_start(tile_ap3[0:PT, 65, :], dram3[PM:P])

    def store_anchor(dram3, tile_ap3):
        nc.sync.dma_start(
            dram3[0:PM].rearrange("(c p) x -> p c x", p=128, c=65),
            tile_ap3[:, 0:65, :])
        nc.sync.dma_start(dram3[PM:P], tile_ap3[0:PT, 65, :])

    with tile.TileContext(nc) as tc, ExitStack() as ctx:
        cpool = ctx.enter_context(tc.tile_pool(name="const", bufs=1))
        big = ctx.enter_context(tc.tile_pool(name="big", bufs=1))
        small = ctx.enter_context(tc.tile_pool(name="small", bufs=1))
        psum = ctx.enter_context(tc.tile_pool(name="psum", bufs=2, space="PSUM"))
        psum1 = ctx.enter_context(tc.tile_pool(name="psum1", bufs=1, space="PSUM"))

        # ---- global constants ----
        ones = cpool.tile([128, 128], dt.float32)
        nc.gpsimd.memset(ones[:], 1.0)
        iden = cpool.tile([128, 128], dt.float32)
        nc.gpsimd.affine_select(iden[:], ones[:], pattern=[[1, 128]], base=0,
                                channel_multiplier=-1,
                                compare_op=Alu.is_equal, fill=0.0)
        clsio_i = cpool.tile([128, C], dt.int32)
        nc.gpsimd.iota(clsio_i[:], pattern=[[1, C]], base=0, channel_multiplier=0)
        clsio = cpool.tile([128, C], dt.float32)
        nc.vector.tensor_copy(clsio[:], clsio_i[:])
        gio_i = cpool.tile([128, G], dt.int32)
        nc.gpsimd.iota(gio_i[:], pattern=[[1, G]], base=0, channel_multiplier=0)
        giof = cpool.tile([128, G], dt.float32)
        nc.vector.tensor_copy(giof[:], gio_i[:])
        grev_i = cpool.tile([128, G], dt.int32)
        nc.gpsimd.iota(grev_i[:], pattern=[[-1, G]], base=G - 1, channel_multiplier=0)
        grev = cpool.tile([128, G], dt.float32)
        nc.vector.tensor_copy(grev[:], grev_i[:])
        infc = cpool.tile([128, 1], dt.float32)
        nc.gpsimd.memset(infc[:], 3.0e38)
        flcol = cpool.tile([128, 1], dt.float32)
        nc.sync.dma_start(flcol[:], fl.rearrange("i g -> (i g)")[:, None])
        flcol8 = cpool.tile([128, 1], dt.uint8)
        nc.vector.tensor_copy(flcol8[:], flcol[:])

        # priors (same for both images): anchor points
        ptst = cpool.tile([128, 2 * CH], dt.float32)
        load_anchor(ptst[:].rearrange("p (c x) -> p c x", x=2), pr[:, 0:2])
        pxv = ptst[:].rearrange("p (c x) -> p x c", x=2)[:, 0]     # [128, 66]
        pyv = ptst[:].rearrange("p (c x) -> p x c", x=2)[:, 1]

        mglob = big.tile([128, CH * 128], dt.float32, tag="mgsc", name="mglob")
        sA = cpool.tile([128, F], dt.float32)             # global scratch
        sB = cpool.tile([128, F], dt.float32)

        per = {}
        for i in range(IMG):
            per[i] = dict(
                iou=big.tile([128, F], dt.float32, tag=f"iou{i}", name=f"iou{i}"),
                alg=big.tile([128, F], dt.float32, tag=f"alg{i}", name=f"alg{i}"),
                ins=big.tile([128, F], dt.uint8, tag=f"ins{i}", name=f"ins{i}"),
            )

        gtt = {}

        # ================= phase A per image =================
        for i in range(IMG):
            pbt = small.tile([128, 4 * CH], dt.float32, tag="pbt")
            load_anchor(pbt[:].rearrange("p (c x) -> p c x", x=4), pb[i])
            pcv = pbt[:].rearrange("p (c x) -> p x c", x=4)
            px1, py1, px2, py2 = pcv[:, 0], pcv[:, 1], pcv[:, 2], pcv[:, 3]

            grow = small.tile([1, 4 * G], dt.float32, tag="grow")
            nc.sync.dma_start(grow[:], gb[i].rearrange("g x -> (g x)")[None, :])
            gbb = small.tile([128, 4 * G], dt.float32, tag=f"gbb{i}")
            nc.gpsimd.partition_broadcast(gbb[:], grow[:])
            gcv = gbb[:].rearrange("p (g x) -> p x g", x=4)
            gx1, gy1, gx2, gy2 = gcv[:, 0], gcv[:, 1], gcv[:, 2], gcv[:, 3]

            lrow = small.tile([1, G], dt.int32, tag="lrow")
            nc.sync.dma_start(lrow[:], gl[i][None, :])
            lbli = small.tile([128, G], dt.int32, tag="lbli")
            nc.gpsimd.partition_broadcast(lbli[:], lrow[:])
            lblf = small.tile([128, G], dt.float32, tag=f"lblf{i}")
            nc.vector.tensor_copy(lblf[:], lbli[:])

            # areas
            areag = small.tile([128, G], dt.float32, tag="areag")
            tw = small.tile([128, G], dt.float32, tag="tw64")
            nc.vector.tensor_tensor(tw[:], gx2, gx1, op=Alu.subtract)
            th = small.tile([128, G], dt.float32, tag="th64")
            nc.vector.tensor_tensor(th[:], gy2, gy1, op=Alu.subtract)
            nc.vector.tensor_tensor(areag[:], tw[:], th[:], op=Alu.mult)
            areap = small.tile([128, CH], dt.float32, tag="areap")
            tw2 = small.tile([128, CH], dt.float32, tag="tw66")
            nc.vector.tensor_tensor(tw2[:], px2, px1, op=Alu.subtract)
            th2 = small.tile([128, CH], dt.float32, tag="th66")
            nc.vector.tensor_tensor(th2[:], py2, py1, op=Alu.subtract)
            nc.vector.tensor_tensor(areap[:], tw2[:], th2[:], op=Alu.mult)

            gtt[i] = dict(gbb=gbb, lblf=lblf)

            # ---- label-gather indices (shared by both score halves) ----
            # idxs[p, k] = (k//4)*80 + lbl[16*(k%4) + p%16], int16, [128, 132]
            lw16 = small.tile([16, 4], dt.int32, tag="lw16")
            nc.sync.dma_start(lw16[:], gl[i].rearrange("(k q) -> q k", q=16))
            lw128 = small.tile([128, 4], dt.int32, tag="lw128")
            for r in range(8):
                nc.sync.dma_start(lw128[16 * r:16 * (r + 1), :], lw16[:])
            idx32 = small.tile([128, 4 * CH2], dt.int32, tag="idx32")
            nc.gpsimd.iota(idx32[:], pattern=[[C, CH2], [0, 4]], base=0,
                           channel_multiplier=0)
            nc.vector.tensor_tensor(
                idx32[:].rearrange("p (c k) -> p c k", k=4),
                idx32[:].rearrange("p (c k) -> p c k", k=4),
                lw128[:, None, :].broadcast_to([128, CH2, 4]),
                op=Alu.add)
            idx16 = small.tile([128, 4 * CH2], dt.int16, tag="idx16")
            nc.vector.tensor_copy(idx16[:], idx32[:])

            # ---- scores gather (two halves) ----
            bs = big.tile([128, F], dt.float32, tag="bufA", name=f"bs{i}")
            psre = ps[i][0:PM].rearrange("(c p) x -> p c x", p=128, c=65)
            for h in range(2):
                sco = big.tile([128, CH2 * C], dt.float32, tag="sc",
                               name=f"sco{i}{h}")
                sc3 = sco[:].rearrange("p (c x) -> p c x", x=C)
                if h == 0:
                    nc.sync.dma_start(sc3[:, :, :], psre[:, 0:33, :])
                else:
                    nc.sync.dma_start(sc3[:, 0:32, :], psre[:, 33:65, :])
                    nc.vector.memset(sc3[:, 32, :], 0.0)
                    nc.sync.dma_start(sc3[0:PT, 32, :], ps[i][PM:P])
                nc.gpsimd.ap_gather(bs[:, h * CH2 * G:(h + 1) * CH2 * G],
                                    sco[:], idx16[:], channels=128,
                                    num_elems=CH2 * C, d=1, num_idxs=CH2 * G)

            # ---- IoU / align / inside chain ----
            # temps: A=sA, B=sB, C=iou, D=alg, E=bufA(after bs dead)
            iou = per[i]["iou"]
            alg = per[i]["alg"]
            ins = per[i]["ins"]
            vA = sA[:].rearrange("p (c g) -> p c g", g=G)
            vB = sB[:].rearrange("p (c g) -> p c g", g=G)
            vC = iou[:].rearrange("p (c g) -> p c g", g=G)
            vD = alg[:].rearrange("p (c g) -> p c g", g=G)

            nc.vector.tensor_tensor(vC, bcC(px2), bcG(gx2), op=Alu.min)
            nc.vector.tensor_tensor(vD, bcC(px1), bcG(gx1), op=Alu.max)
            nc.vector.tensor_tensor(iou[:], iou[:], alg[:], op=Alu.subtract)  # ix
            nc.vector.tensor_tensor(vA, bcC(py2), bcG(gy2), op=Alu.min)
            nc.vector.tensor_tensor(vB, bcC(py1), bcG(gy1), op=Alu.max)
            nc.vector.tensor_tensor(sA[:], sA[:], sB[:], op=Alu.subtract)     # iy
            nc.vector.tensor_scalar(iou[:], iou[:], 0.0, None, op0=Alu.max)
            nc.vector.scalar_tensor_tensor(sB[:], sA[:], 0.0, iou[:],
                                           op0=Alu.max, op1=Alu.mult)   # inter
            nc.vector.tensor_tensor(vA, bcG(areag[:]), bcC(areap[:]), op=Alu.add)
            nc.vector.tensor_tensor(sA[:], sA[:], sB[:], op=Alu.subtract)  # union
            nc.vector.reciprocal_approx_fast(sA[:], sA[:])
            nc.vector.tensor_tensor(iou[:], sB[:], sA[:], op=Alu.mult)    # iou

            nc.scalar.square(sA[:], iou[:])            # o2 (ACT)
            nc.scalar.square(sB[:], sA[:])             # o4 (ACT)
            nc.gpsimd.tensor_tensor(sA[:], sB[:], sA[:], op=Alu.mult)     # o6
            nc.vector.tensor_tensor(alg[:], sA[:], bs[:], op=Alu.mult)    # align

            etmp = big.tile([128, F], dt.float32, tag="bufA", name=f"etmp{i}")
            vE = etmp[:].rearrange("p (c g) -> p c g", g=G)
            nc.vector.tensor_tensor(vA, bcC(pxv), bcG(gx1), op=Alu.subtract)
            nc.gpsimd.tensor_tensor(vB, bcG(gx2), bcC(pxv), op=Alu.subtract)
            nc.vector.tensor_tensor(sA[:], sA[:], sB[:], op=Alu.min)
            nc.vector.tensor_tensor(vB, bcC(pyv), bcG(gy1), op=Alu.subtract)
            nc.gpsimd.tensor_tensor(vE, bcG(gy2), bcC(pyv), op=Alu.subtract)
            nc.vector.tensor_tensor(sB[:], sB[:], etmp[:], op=Alu.min)
            nc.vector.tensor_tensor(sA[:], sA[:], sB[:], op=Alu.min)      # mins
            nc.vector.tensor_scalar(sB[:], sA[:], IOU_EPS, None, op0=Alu.is_gt)
            nc.scalar.copy(ins[:], sB[:])                                  # u8
            nc.vector.tensor_tensor(sB[:], sB[:], alg[:], op=Alu.mult)    # metrics

            if taps and i == 0:
                nc.sync.dma_start(didx, idx16[:])
                nc.sync.dma_start(dlw, lw128[:])
                nc.sync.dma_start(dbs, bs[:])
                nc.sync.dma_start(diou, iou[:])
                nc.sync.dma_start(dalg, alg[:])
                nc.sync.dma_start(dins, ins[:])
                nc.sync.dma_start(dmet, sA[:])

            # ---- PE transpose metrics to gt-major ----
            groups = [(c0, min(8, CH - c0)) for c0 in range(0, CH, 8)]
            for (c0, n) in groups:
                pt = psum.tile([G, 8 * 128], dt.float32)
                for c in range(n):
                    nc.tensor.transpose(
                        pt[:, c * 128:(c + 1) * 128], vB[:, c0 + c, :], iden[:])
                nc.scalar.copy(mglob[i * G:(i + 1) * G, c0 * 128:(c0 + n) * 128],
                               pt[:, 0:n * 128])

        # ================= top-13 threshold =================
        m8a = small.tile([128, 8], dt.float32, tag="m8a")
        nc.vector.max(m8a[:], mglob[:])
        nc.vector.match_replace(mglob[:], m8a[:], mglob[:], imm_value=-1.0)
        m8b = small.tile([128, 8], dt.float32, tag="m8b")
        nc.vector.max(m8b[:], mglob[:])
        t13 = small.tile([128, 1], dt.float32, tag="t13")
        nc.vector.tensor_scalar(t13[:], m8b[:, 4:5], TINY, None, op0=Alu.max)
        t13s = small.tile([128, 1], dt.float32, tag="t13s")
        nc.vector.select(t13s[:], flcol8[:], t13[:], infc[:])
        ptr = psum1.tile([1, 128], dt.float32)
        nc.tensor.transpose(ptr[:], t13s[:], iden[:])
        t13r = small.tile([1, 128], dt.float32, tag="t13r")
        nc.scalar.copy(t13r[:], ptr[:])
        t13b = small.tile([128, 128], dt.float32, tag="t13b")
        nc.gpsimd.partition_broadcast(t13b[:], t13r[:])
        if taps:
            nc.sync.dma_start(dmg, mglob[:])
            nc.sync.dma_start(dt13, t13b[:])

        # ================= resolution + outputs per image =================
        for i in range(IMG):
            iou = per[i]["iou"]
            alg = per[i]["alg"]
            ins = per[i]["ins"]
            gbb = gtt[i]["gbb"]
            lblf = gtt[i]["lblf"]
            gcv = gbb[:].rearrange("p (g x) -> p x g", x=4)
            vA = sA[:].rearrange("p (c g) -> p c g", g=G)
            vB = sB[:].rearrange("p (c g) -> p c g", g=G)
            algv = alg[:].rearrange("p (c g) -> p c g", g=G)
            iouv = iou[:].rearrange("p (c g) -> p c g", g=G)

            ppre = big.tile([128, F], dt.float32, tag="bufA", name=f"ppre{i}")
            pv = ppre[:].rearrange("p (c g) -> p c g", g=G)
            tb = t13b[:, i * G:(i + 1) * G]
            nc.vector.tensor_tensor(pv, algv, bcG(tb), op=Alu.is_ge)
            nc.vector.tensor_tensor(ppre[:], ppre[:], ins[:], op=Alu.mult)

            fgpre = small.tile([128, CH], dt.float32, tag="fgpre")
            nc.vector.tensor_reduce(fgpre[:], pv, axis=Ax.X, op=Alu.add)
            colmax = small.tile([128, CH], dt.float32, tag="colmax")
            nc.vector.tensor_reduce(colmax[:], iouv, axis=Ax.X, op=Alu.max)
            nc.vector.tensor_tensor(vA, iouv, bcC(colmax[:]), op=Alu.is_equal)
            nc.gpsimd.tensor_tensor(vA, vA, bcG(grev[:]), op=Alu.mult)
            argg = small.tile([128, CH], dt.float32, tag="argg")
            nc.vector.tensor_reduce(argg[:], vA, axis=Ax.X, op=Alu.max)
            nc.vector.tensor_scalar(argg[:], argg[:], -1.0, float(G - 1),
                                    op0=Alu.mult, op1=Alu.add)
            multi = small.tile([128, CH], dt.float32, tag="multi")
            nc.vector.tensor_scalar(multi[:], fgpre[:], 1.0, None, op0=Alu.is_gt)
            code = small.tile([128, CH], dt.float32, tag="code")
            nc.vector.scalar_tensor_tensor(code[:], argg[:], 1.0, multi[:],
                                           op0=Alu.add, op1=Alu.mult)
            nc.vector.tensor_scalar(code[:], code[:], -1.0, None, op0=Alu.add)
            nmask = small.tile([128, CH], dt.float32, tag="nmask")
            nc.vector.tensor_scalar(nmask[:], code[:], 0.0, None, op0=Alu.is_lt)
            nc.vector.tensor_tensor(vB, bcC(code[:]), bcG(giof[:]),
                                    op=Alu.is_equal)
            nc.vector.tensor_tensor(pv, pv, bcC(nmask[:]), op=Alu.mult)
            nc.vector.tensor_tensor(ppre[:], ppre[:], sB[:], op=Alu.add)  # pos
            pos = ppre
            pv = pos[:].rearrange("p (c g) -> p c g", g=G)

            if taps and i == 0:
                nc.sync.dma_start(dpos, pos[:])
            fg = small.tile([128, CH], dt.float32, tag=f"fg{i}")
            nc.vector.tensor_reduce(fg[:], pv, axis=Ax.X, op=Alu.add)

            nc.vector.tensor_tensor(sA[:], alg[:], pos[:], op=Alu.mult)   # am
            nc.gpsimd.tensor_tensor(sB[:], iou[:], pos[:], op=Alu.mult)   # ovl*pos
            pac = small.tile([128, G], dt.float32, tag="pac")
            nc.vector.tensor_reduce(
                pac[:], sA[:].rearrange("p (c g) -> p g c", g=G), axis=Ax.X,
                op=Alu.max)
            paa = small.tile([128, G], dt.float32, tag="paa")
            nc.gpsimd.partition_all_reduce(paa[:], pac[:], channels=128,
                                           reduce_op=bass_isa.ReduceOp.max)
            poc = small.tile([128, G], dt.float32, tag="poc")
            nc.vector.tensor_reduce(
                poc[:], sB[:].rearrange("p (c g) -> p g c", g=G), axis=Ax.X,
                op=Alu.max)
            poa = small.tile([128, G], dt.float32, tag="poa")
            nc.gpsimd.partition_all_reduce(poa[:], poc[:], channels=128,
                                           reduce_op=bass_isa.ReduceOp.max)
            nc.vector.tensor_scalar(paa[:], paa[:], EPS, None, op0=Alu.add)
            cgr = small.tile([128, G], dt.float32, tag="cgr")
            nc.vector.reciprocal(cgr[:], paa[:])
            cg = small.tile([128, G], dt.float32, tag="cg")
            nc.vector.tensor_tensor(cg[:], poa[:], cgr[:], op=Alu.mult)

            # norm (am still in sA; in-place scale by cg then reduce)
            nc.vector.tensor_tensor(vA, vA, bcG(cg[:]), op=Alu.mult)
            normv = small.tile([128, CH], dt.float32, tag=f"normv{i}")
            nc.vector.tensor_reduce(normv[:], vA, axis=Ax.X, op=Alu.add)

            # labels
            fgf = small.tile([128, CH], dt.float32, tag="fgf")
            nc.vector.tensor_scalar(fgf[:], fg[:], 0.0, None, op0=Alu.is_gt)
            fgb8 = small.tile([128, CH], dt.uint8, tag="fgb8")
            nc.vector.tensor_copy(fgb8[:], fgf[:])
            nc.vector.tensor_tensor(vA, pv, bcG(lblf[:]), op=Alu.mult)
            lsum = small.tile([128, CH], dt.float32, tag="lsum")
            nc.vector.tensor_reduce(lsum[:], vA, axis=Ax.X, op=Alu.add)
            labf = small.tile([128, CH], dt.float32, tag=f"labf{i}")
            nc.vector.select(labf[:], fgb8[:], lsum[:],
                             lblf[:, 0:1].broadcast_to([128, CH]))

            # bboxes
            bbo = small.tile([128, 4 * CH], dt.float32, tag="bbo")
            bbv = bbo[:].rearrange("p (c x) -> p x c", x=4)
            for j in range(4):
                eng, tgt = (nc.vector, vA) if j % 2 == 0 else (nc.gpsimd, vB)
                eng.tensor_tensor(tgt, pv, bcG(gcv[:, j]), op=Alu.mult)
                bsum = small.tile([128, CH], dt.float32, tag="bsum")
                nc.vector.tensor_reduce(bsum[:], tgt, axis=Ax.X, op=Alu.add)
                nc.vector.select(bbv[:, j], fgb8[:], bsum[:],
                                 gbb[:, j:j + 1].broadcast_to([128, CH]))

            # labels cast
            labi = small.tile([128, CH], dt.int32, tag="labi")
            nc.vector.tensor_copy(labi[:], labf[:])

            # scores dense one-hot * norm
            sc_out = big.tile([128, CH * C], dt.float32, tag="mgsc",
                              name=f"sc_out{i}")
            scv = sc_out[:].rearrange("p (c x) -> p c x", x=C)
            nc.vector.tensor_tensor(
                scv,
                clsio[:, None, :].broadcast_to([128, CH, C]),
                labf[:, :, None].broadcast_to([128, CH, C]),
                op=Alu.is_equal)
            nc.gpsimd.tensor_tensor(
                scv, scv,
                normv[:, :, None].broadcast_to([128, CH, C]),
                op=Alu.mult)

            # ---- outputs ----
            store_anchor(olab[i][:, None], labi[:, :, None])
            store_anchor(ofg[i][:, None], fgb8[:, :, None])
            store_anchor(obb[i], bbo[:].rearrange("p (c x) -> p c x", x=4))
            store_anchor(osc[i], scv)

    nc.compile()
    return nc


def _get_nc():
    if "nc" not in _CACHE:
        _CACHE["nc"] = _build()
    return _CACHE["nc"]


def kernel(pred_bboxes, pred_scores, priors, gt_labels, gt_bboxes, pad_bbox_flag):
    from concourse.bass_utils import run_bass_kernel_spmd

    pred_bboxes = np.ascontiguousarray(np.asarray(pred_bboxes, dtype=np.float32))
    pred_scores = np.ascontiguousarray(np.asarray(pred_scores, dtype=np.float32))
    priors = np.ascontiguousarray(np.asarray(priors, dtype=np.float32))
    lab_dt = np.asarray(gt_labels).dtype
    gli = np.ascontiguousarray(np.asarray(gt_labels).reshape(-1, G).astype(np.int32))
    gbf = np.ascontiguousarray(np.asarray(gt_bboxes, dtype=np.float32))
    flf = np.ascontiguousarray(
        np.asarray(pad_bbox_flag, dtype=np.float32).reshape(-1, G))

    B = pred_scores.shape[0]
    ncores = 8
    per = B // ncores
    assert per == IMG

    nc = _get_nc()
    in_maps = []
    for c in range(ncores):
        s = slice(c * per, (c + 1) * per)
        in_maps.append({
            "pb": pred_bboxes[s],
            "ps": pred_scores[s],
            "pr": priors,
            "gl": gli[s],
            "gb": gbf[s],
            "fl": flf[s],
        })
    res = run_bass_kernel_spmd(nc, in_maps, core_ids=list(range(ncores)))
    labs = np.concatenate([r["olab"] for r in res.results], axis=0)
    bbs = np.concatenate([r["obb"] for r in res.results], axis=0)
    scs = np.concatenate([r["osc"] for r in res.results], axis=0)
    fgs = np.concatenate([r["ofg"] for r in res.results], axis=0)
    return (labs.astype(lab_dt), bbs, scs, fgs.astype(bool))
